# revision 12
# baseline (speedup 1.0000x reference)
"""Multi-head attention forward on 8 TRN2 NeuronCores, data-parallel over batch.

Reference computation (per batch element b):
    qkv  = x @ qkv_w.T + qkv_b                     # [N, 3D]
    q, k = LN_headdim(q), LN_headdim(k)            # layernorm over head_dim=64
    S    = q @ k.T * hd^-0.5 ; A = softmax_j(S)    # per head
    out  = (A @ v) @ proj_w.T + proj_b             # [N, D]

Kernel strategy (one batch element per core, no collectives):
  - bf16 matmuls on TensorE; f32 statistics/softmax denominators.
  - Scores computed TRANSPOSED: ST[j,i] = k_j . q_i so that E = exp(ST*scale)
    lands in SBUF with the contraction axis j on partitions -- E is directly
    the lhsT of the attn@v matmul (no attention-matrix transpose needed).
  - Softmax denominators come free: V gets a ones-column appended, so
    psum[i, 64] = sum_j E[j,i]; normalize with a per-partition scalar.
  - No max-subtraction in softmax: q,k are layernormed so |q.k|*scale <= 8,
    exp() is safely bounded (<= e^8) in f32/bf16.
  - All transposes on TensorE (identity matmul), batched 4 tiles into one
    [128,512] PSUM bank with a single evacuation copy.  DMA transposes are
    avoided entirely: they shatter into 256B packets (measured 780us of DMA
    engine time for this problem).
  - Engine balance: exp on ScalarE; reductions/psum-reads on VectorE;
    SBUF-only elementwise (casts, squares, LN scale) on GpSimd.
"""

import sys

import numpy as np

sys.path.insert(0, "/opt/trn_rl_repo")

from contextlib import ExitStack

import concourse.bass as bass
import concourse.tile as tile
from concourse import bacc, mybir
from concourse.bass_utils import run_bass_kernel_spmd
from concourse.masks import make_identity

B, N, D = 8, 1024, 768
H, HD = 12, 64
O3 = 3 * D  # 2304
P = 128
NT = N // P  # 8 token tiles
DC = D // P  # 6 contraction subtiles
EPS = 1e-5
SCALE = HD ** -0.5  # 0.125
F32 = mybir.dt.float32
BF16 = mybir.dt.bfloat16

# qkv output chunks: [start, size]; q = o[0:768), k = [768:1536), v = [1536:2304)
QKV_CHUNKS = [(0, 512), (512, 512), (1024, 512), (1536, 512), (2048, 256)]


def _bcast_ap(ap_1d, parts):
    """View a 1-D DRAM AP as [parts, n] with partition stride 0 (broadcast)."""
    return bass.AP(
        tensor=ap_1d.tensor,
        offset=ap_1d.offset,
        ap=[[0, parts]] + list(ap_1d.ap),
    )


def _groups_of(n, g):
    """Split range(n) into [(start, len)] groups of at most g."""
    return [(s, min(g, n - s)) for s in range(0, n, g)]


def _build_graph(apply_gn):
    nc = bacc.Bacc("TRN2", target_bir_lowering=False, debug=False, num_devices=B)

    x_d = nc.dram_tensor("x", [N, D], F32, kind="ExternalInput").ap()
    qkvw_d = nc.dram_tensor("qkv_w", [O3, D], F32, kind="ExternalInput").ap()
    qkvb_d = nc.dram_tensor("qkv_b", [O3], F32, kind="ExternalInput").ap()
    projw_d = nc.dram_tensor("proj_w", [D, D], F32, kind="ExternalInput").ap()
    projb_d = nc.dram_tensor("proj_b", [D], F32, kind="ExternalInput").ap()
    gamma_d = nc.dram_tensor("qn_gamma", [HD], F32, kind="ExternalInput").ap()
    beta_d = nc.dram_tensor("qn_beta", [HD], F32, kind="ExternalInput").ap()
    out_d = nc.dram_tensor("out", [N, D], F32, kind="ExternalOutput").ap()

    with tile.TileContext(nc) as tc:
        _emit(tc, out_d, x_d, qkvw_d, qkvb_d, projw_d, projb_d, gamma_d, beta_d,
              apply_gn)

    nc.compile()
    return nc


def _emit(tc, out_d, x_d, qkvw_d, qkvb_d, projw_d, projb_d, gamma_d, beta_d,
          apply_gn):
    nc = tc.nc
    ctx = ExitStack()
    with ctx:
        const = ctx.enter_context(tc.tile_pool(name="const", bufs=1))
        wpool = ctx.enter_context(tc.tile_pool(name="wts", bufs=1))
        data = ctx.enter_context(tc.tile_pool(name="data", bufs=1))
        epool = ctx.enter_context(tc.tile_pool(name="escore", bufs=2))
        qkpool = ctx.enter_context(tc.tile_pool(name="qk", bufs=2))
        tmpp = ctx.enter_context(tc.tile_pool(name="tmp", bufs=3))
        stat = ctx.enter_context(tc.tile_pool(name="stat", bufs=4))
        outp = ctx.enter_context(tc.tile_pool(name="outp", bufs=3))
        nrm = ctx.enter_context(tc.tile_pool(name="nrm", bufs=2))

        # ---- constants ----
        qkvb_bc = const.tile([P, O3], F32)
        nc.sync.dma_start(qkvb_bc[:], _bcast_ap(qkvb_d, P))
        projb_bc = const.tile([P, D], F32)
        nc.sync.dma_start(projb_bc[:], _bcast_ap(projb_d, P))
        eps_t = const.tile([P, 1], F32)
        nc.vector.memset(eps_t[:], EPS)
        ident = const.tile([P, P], BF16)
        make_identity(nc, ident[:])
        if apply_gn:
            gamma_bc = const.tile([P, HD], F32)
            nc.sync.dma_start(gamma_bc[:], _bcast_ap(gamma_d, P))
            beta_bc = const.tile([P, HD], F32)
            nc.sync.dma_start(beta_bc[:], _bcast_ap(beta_d, P))

        def pe_transpose_batch(pool, src_tiles, dst, dst_col0, evac_engine):
            """PE-transpose up to 4 [128,128] bf16 tiles through one PSUM bank;
            dst gets columns [dst_col0, dst_col0 + 128*len)."""
            ng = len(src_tiles)
            ps_full = pool.tile([P, 512], BF16, tag="tr", name="ps_tr_t")
            ps = ps_full[:, :ng * P]
            for i, src in enumerate(src_tiles):
                nc.tensor.transpose(ps_full[:, i * P:(i + 1) * P], src, ident[:])
            if evac_engine is nc.scalar:
                evac_engine.copy(dst[:, dst_col0:dst_col0 + ng * P], ps)
            else:
                evac_engine.tensor_copy(dst[:, dst_col0:dst_col0 + ng * P], ps)

        # ---- load + cast + PE-transpose x and weights into [k, ., m] layouts ----
        xT = wpool.tile([P, DC, N], BF16)      # [d_in, d_out, t]
        qkvwT = wpool.tile([P, DC, O3], BF16)  # [d_in, d_out, o]
        projwT = wpool.tile([P, DC, D], BF16)  # [o_in, o_out, e]

        # ---- phase 0+1 PSUM scope: transposes + qkv matmuls ----
        qn = data.tile([P, NT, D], BF16)            # [t_in, t_out, o]  (q heads)
        kn = data.tile([P, NT, D], BF16)
        # v with 64 ones-columns: attn@v psum rows 64:128 become the softmax
        # denominator s[i], broadcast across 64 partitions by the PE for free
        vext = data.tile([P, NT, H, 2 * HD], BF16)
        nc.vector.memset(vext[:, :, :, HD:2 * HD], 1.0)

        with tc.tile_pool(name="ps_tr", bufs=2, space="PSUM") as ps_tr, \
             tc.tile_pool(name="ps_mm", bufs=4, space="PSUM") as ps_mm, \
             tc.tile_pool(name="prep", bufs=2) as prep:

            def load_cast_transpose(src_d, n_rt, dstT):
                for gi, (g0, gn) in enumerate(_groups_of(n_rt, 4)):
                    stage = prep.tile([P, 4, D], BF16, tag="stage", name="stage")
                    for i in range(gn):
                        t_f = prep.tile([P, D], F32, tag="ld_f32", name="t_f")
                        nc.sync.dma_start(
                            t_f[:], src_d[(g0 + i) * P:(g0 + i + 1) * P, :]
                        )
                        nc.vector.tensor_copy(stage[:, i, :], t_f[:])
                    for dc in range(DC):
                        srcs = [stage[:, i, dc * P:(dc + 1) * P]
                                for i in range(gn)]
                        evac = nc.scalar if (gi + dc) % 2 == 0 else nc.vector
                        pe_transpose_batch(ps_tr, srcs, dstT[:, dc, :], g0 * P,
                                           evac)

            load_cast_transpose(x_d, NT, xT)
            load_cast_transpose(qkvw_d, O3 // P, qkvwT)
            load_cast_transpose(projw_d, D // P, projwT)

            # ---- QKV projection + bias + head-dim layernorm on q,k ----
            for tt in range(NT):
                for (c0, cs) in QKV_CHUNKS:
                    psum_full = ps_mm.tile([P, 512], F32, tag="mm", name="psum_mm")
                    psum = psum_full[:, :cs]
                    for dc in range(DC):
                        nc.tensor.matmul(
                            psum,
                            lhsT=xT[:, dc, tt * P:(tt + 1) * P],
                            rhs=qkvwT[:, dc, c0:c0 + cs],
                            start=(dc == 0),
                            stop=(dc == DC - 1),
                        )
                    if c0 < 2 * D:
                        # q/k chunk: bias add then LN over 64-wide segments
                        nsg = cs // HD
                        tmp_c_full = tmpp.tile([P, 512], F32, tag="tmpc", name="tmp_c")
                        tmp_c = tmp_c_full[:, :cs]
                        nc.vector.tensor_add(tmp_c, psum, qkvb_bc[:, c0:c0 + cs])
                        t3 = tmp_c.rearrange("p (s h) -> p s h", h=HD)
                        sums_full = stat.tile([P, 8], F32, tag="sums", name="sums")
                        sums = sums_full[:, :nsg]
                        nc.vector.tensor_reduce(
                            sums, t3, axis=mybir.AxisListType.X,
                            op=mybir.AluOpType.add
                        )
                        sq_full = tmpp.tile([P, 512], F32, tag="sq", name="sq")
                        sq = sq_full[:, :cs]
                        nc.scalar.square(sq, tmp_c)
                        sqs_full = stat.tile([P, 8], F32, tag="sqs", name="sqs")
                        sqs = sqs_full[:, :nsg]
                        nc.vector.tensor_reduce(
                            sqs,
                            sq.rearrange("p (s h) -> p s h", h=HD),
                            axis=mybir.AxisListType.X,
                            op=mybir.AluOpType.add,
                        )
                        mean_full = stat.tile([P, 8], F32, tag="mean", name="mean")
                        mean = mean_full[:, :nsg]
                        nc.vector.tensor_scalar_mul(mean, sums, 1.0 / HD)
                        msq_full = stat.tile([P, 8], F32, tag="msq", name="msq")
                        msq = msq_full[:, :nsg]
                        nc.vector.tensor_mul(msq, mean, mean)
                        var_full = stat.tile([P, 8], F32, tag="var", name="var")
                        var = var_full[:, :nsg]
                        nc.vector.tensor_scalar_mul(var, sqs, 1.0 / HD)
                        nc.vector.tensor_sub(var, var, msq)
                        std_full = stat.tile([P, 8], F32, tag="std", name="std")
                        std = std_full[:, :nsg]
                        nc.scalar.activation(
                            std, var, mybir.ActivationFunctionType.Sqrt,
                            bias=eps_t[:]
                        )
                        rstd_full = stat.tile([P, 8], F32, tag="rstd", name="rstd")
                        rstd = rstd_full[:, :nsg]
                        nc.vector.reciprocal(rstd, std)
                        # normalize: (tmp - mean) * rstd  (broadcast stats over HD)
                        mean_b = mean[:, :, None].to_broadcast((P, nsg, HD))
                        rstd_b = rstd[:, :, None].to_broadcast((P, nsg, HD))
                        nc.gpsimd.tensor_tensor(t3, t3, mean_b,
                                                op=mybir.AluOpType.subtract)
                        if apply_gn:
                            nc.gpsimd.tensor_tensor(t3, t3, rstd_b,
                                                    op=mybir.AluOpType.mult)
                            gamma_b = gamma_bc[:, None, :].to_broadcast((P, nsg, HD))
                            nc.gpsimd.tensor_tensor(t3, t3, gamma_b,
                                                    op=mybir.AluOpType.mult)
                        # write bf16 into qn/kn, splitting at q/k boundary (o=768)
                        spans = []
                        if c0 < D:
                            q_hi = min(c0 + cs, D)
                            spans.append((qn, c0, q_hi - c0, 0))
                        if c0 + cs > D:
                            k_lo = max(c0, D)
                            spans.append((kn, k_lo - D, c0 + cs - k_lo, k_lo - c0))
                        for (dst, d0, dlen, src_off) in spans:
                            nsg_s = dlen // HD
                            src = t3[:, src_off // HD:(src_off + dlen) // HD, :]
                            dgt = dst[:, tt, d0:d0 + dlen].rearrange(
                                "p (s h) -> p s h", h=HD
                            )
                            if apply_gn:
                                beta_b = beta_bc[:, None, :].to_broadcast(
                                    (P, nsg_s, HD))
                                nc.gpsimd.tensor_tensor(
                                    dgt, src, beta_b, op=mybir.AluOpType.add
                                )
                            else:
                                rstd_s = rstd_b[:, src_off // HD:
                                                (src_off + dlen) // HD, :]
                                nc.gpsimd.tensor_tensor(
                                    dgt, src, rstd_s, op=mybir.AluOpType.mult
                                )
                    else:
                        # v chunk: bias add, cast bf16, scatter into vext
                        hs = (c0 - 2 * D) // HD
                        nh = cs // HD
                        nc.vector.tensor_tensor(
                            vext[:, tt, hs:hs + nh, 0:HD],
                            psum.rearrange("p (s h) -> p s h", h=HD),
                            qkvb_bc[:, c0:c0 + cs].rearrange(
                                "p (s h) -> p s h", h=HD),
                            op=mybir.AluOpType.add,
                        )

        # ---- per-head attention ----
        # attnoutT [o_in, o_out, t] is written directly by the normalize step
        attnoutT = data.tile([P, DC, N], BF16)
        with tc.tile_pool(name="ps_tr2", bufs=2, space="PSUM") as ps_tr2, \
             tc.tile_pool(name="ps_st", bufs=2, space="PSUM") as ps_st, \
             tc.tile_pool(name="ps_av", bufs=2, space="PSUM") as ps_av:

            def emit_pair_transposes(hp):
                # qqT/kkT: [hd, t]; head 2hp in partitions 0:64, 2hp+1 in 64:128
                qqT = qkpool.tile([P, N], BF16, tag="qqT", name="qqT")
                kkT = qkpool.tile([P, N], BF16, tag="kkT", name="kkT")
                for (g0, gn) in _groups_of(NT, 4):
                    srcs_q = [qn[:, g0 + i, hp * P:(hp + 1) * P] for i in range(gn)]
                    pe_transpose_batch(ps_tr2, srcs_q, qqT, g0 * P, nc.scalar)
                    srcs_k = [kn[:, g0 + i, hp * P:(hp + 1) * P] for i in range(gn)]
                    pe_transpose_batch(ps_tr2, srcs_k, kkT, g0 * P, nc.scalar)
                return qqT, kkT

            cur = emit_pair_transposes(0)
            for hp in range(H // 2):
                qqT, kkT = cur
                for hh in range(2):
                    h = hp * 2 + hh
                    qT = qqT[hh * HD:(hh + 1) * HD, :]
                    kT = kkT[hh * HD:(hh + 1) * HD, :]
                    # E[j, i] = exp(scale * k_j . q_i); one exp per j-tile
                    E = epool.tile([P, NT, N], BF16, tag="E")
                    for jt in range(NT):
                        ps = ps_st.tile([P, N], F32, tag="st", name="ps_st_t")
                        for ic in range(2):
                            nc.tensor.matmul(
                                ps[:, ic * 512:(ic + 1) * 512],
                                lhsT=kT[:, jt * P:(jt + 1) * P],
                                rhs=qT[:, ic * 512:(ic + 1) * 512],
                                start=True,
                                stop=True,
                            )
                        nc.scalar.activation(
                            E[:, jt, :],
                            ps,
                            mybir.ActivationFunctionType.Exp,
                            scale=SCALE,
                        )
                    # psum[0:64, i]  = sum_j v[j, h'] E[j, i]
                    # psum[64:128, i] = s[i] (denominator, PE-broadcast by the
                    # 64 ones-columns of vext); both i-halves interleaved so the
                    # 16 matmuls run back-to-back once E is ready
                    pa0 = ps_av.tile([P, 512], F32, tag="av", name="pa0")
                    pa1 = ps_av.tile([P, 512], F32, tag="av", name="pa1")
                    for jt in range(NT):
                        nc.tensor.matmul(
                            pa0,
                            lhsT=vext[:, jt, h, :],
                            rhs=E[:, jt, 0:512],
                            start=(jt == 0),
                            stop=(jt == NT - 1),
                        )
                        nc.tensor.matmul(
                            pa1,
                            lhsT=vext[:, jt, h, :],
                            rhs=E[:, jt, 512:1024],
                            start=(jt == 0),
                            stop=(jt == NT - 1),
                        )
                    for ic, pa in ((0, pa0), (1, pa1)):
                        rcp_t = nrm.tile([HD, 512], F32, tag="rcp_t", name="rcp_t")
                        nc.vector.reciprocal(rcp_t[:], pa[HD:2 * HD, :])
                        nc.vector.tensor_tensor(
                            attnoutT[(h % 2) * HD:(h % 2 + 1) * HD, h // 2,
                                     ic * 512:(ic + 1) * 512],
                            pa[0:HD, :],
                            rcp_t[:],
                            op=mybir.AluOpType.mult,
                        )
                if hp + 1 < H // 2:
                    cur = emit_pair_transposes(hp + 1)

        # ---- output projection ----
        EC = 384
        with tc.tile_pool(name="ps_pj", bufs=2, space="PSUM") as ps_pj:
            for tt in range(NT):
                for ec in range(D // EC):
                    ps_full = ps_pj.tile([P, 512], F32, tag="mm", name="ps_proj")
                    ps = ps_full[:, :EC]
                    for oc in range(DC):
                        nc.tensor.matmul(
                            ps,
                            lhsT=attnoutT[:, oc, tt * P:(tt + 1) * P],
                            rhs=projwT[:, oc, ec * EC:(ec + 1) * EC],
                            start=(oc == 0),
                            stop=(oc == DC - 1),
                        )
                    ot = outp.tile([P, EC], F32, tag="outt")
                    nc.vector.tensor_add(ot[:], ps,
                                         projb_bc[:, ec * EC:(ec + 1) * EC])
                    nc.sync.dma_start(
                        out_d[tt * P:(tt + 1) * P, ec * EC:(ec + 1) * EC], ot[:]
                    )


_NC_CACHE = {}


def _get_nc(apply_gn=True):
    if apply_gn not in _NC_CACHE:
        _NC_CACHE[apply_gn] = _build_graph(apply_gn)
    return _NC_CACHE[apply_gn]


def kernel(x, qkv_w, qkv_b, proj_w, proj_b, qn_gamma, qn_beta):
    qn_gamma = np.ascontiguousarray(qn_gamma, np.float32)
    qn_beta = np.ascontiguousarray(qn_beta, np.float32)
    apply_gn = not (np.all(qn_gamma == 1.0) and np.all(qn_beta == 0.0))
    nc = _get_nc(apply_gn)
    shared = {
        "qkv_w": np.ascontiguousarray(qkv_w, np.float32),
        "qkv_b": np.ascontiguousarray(qkv_b, np.float32),
        "proj_w": np.ascontiguousarray(proj_w, np.float32),
        "proj_b": np.ascontiguousarray(proj_b, np.float32),
        "qn_gamma": qn_gamma,
        "qn_beta": qn_beta,
    }
    in_maps = [
        {**shared, "x": np.ascontiguousarray(x[i], np.float32)} for i in range(B)
    ]
    res = run_bass_kernel_spmd(nc, in_maps, core_ids=list(range(B)))
    return np.stack([res.results[i]["out"] for i in range(B)], axis=0)


# revision 13
# speedup vs baseline: 1.1812x; 1.1812x over previous
"""Multi-head attention forward on 8 TRN2 NeuronCores, data-parallel over batch.

Reference computation (per batch element b):
    qkv  = x @ qkv_w.T + qkv_b                     # [N, 3D]
    q, k = LN_headdim(q), LN_headdim(k)            # layernorm over head_dim=64
    S    = q @ k.T * hd^-0.5 ; A = softmax_j(S)    # per head
    out  = (A @ v) @ proj_w.T + proj_b             # [N, D]

Kernel strategy (one batch element per core, no collectives):
  - bf16 matmuls on TensorE; f32 statistics/softmax denominators.
  - Scores computed TRANSPOSED: ST[j,i] = k_j . q_i so that E = exp(ST*scale)
    lands in SBUF with the contraction axis j on partitions -- E is directly
    the lhsT of the attn@v matmul (no attention-matrix transpose needed).
  - Softmax denominators come free: V gets a ones-column appended, so
    psum[i, 64] = sum_j E[j,i]; normalize with a per-partition scalar.
  - No max-subtraction in softmax: q,k are layernormed so |q.k|*scale <= 8,
    exp() is safely bounded (<= e^8) in f32/bf16.
  - All transposes on TensorE (identity matmul), batched 4 tiles into one
    [128,512] PSUM bank with a single evacuation copy.  DMA transposes are
    avoided entirely: they shatter into 256B packets (measured 780us of DMA
    engine time for this problem).
  - Engine balance: exp on ScalarE; reductions/psum-reads on VectorE;
    SBUF-only elementwise (casts, squares, LN scale) on GpSimd.
"""

import sys

import numpy as np

sys.path.insert(0, "/opt/trn_rl_repo")

from contextlib import ExitStack

import concourse.bass as bass
import concourse.tile as tile
from concourse import bacc, mybir
from concourse.bass_utils import run_bass_kernel_spmd
from concourse.masks import make_identity

B, N, D = 8, 1024, 768
H, HD = 12, 64
O3 = 3 * D  # 2304
P = 128
NT = N // P  # 8 token tiles
DC = D // P  # 6 contraction subtiles
EPS = 1e-5
SCALE = HD ** -0.5  # 0.125
F32 = mybir.dt.float32
BF16 = mybir.dt.bfloat16

# qkv output chunks: [start, size]; q = o[0:768), k = [768:1536), v = [1536:2304)
QKV_CHUNKS = [(0, 512), (512, 512), (1024, 512), (1536, 512), (2048, 256)]


def _bcast_ap(ap_1d, parts):
    """View a 1-D DRAM AP as [parts, n] with partition stride 0 (broadcast)."""
    return bass.AP(
        tensor=ap_1d.tensor,
        offset=ap_1d.offset,
        ap=[[0, parts]] + list(ap_1d.ap),
    )


def _groups_of(n, g):
    """Split range(n) into [(start, len)] groups of at most g."""
    return [(s, min(g, n - s)) for s in range(0, n, g)]


def _build_graph(apply_gn):
    nc = bacc.Bacc("TRN2", target_bir_lowering=False, debug=False, num_devices=B)

    x_d = nc.dram_tensor("x", [N, D], F32, kind="ExternalInput").ap()
    qkvw_d = nc.dram_tensor("qkv_w", [O3, D], F32, kind="ExternalInput").ap()
    qkvb_d = nc.dram_tensor("qkv_b", [O3], F32, kind="ExternalInput").ap()
    projw_d = nc.dram_tensor("proj_w", [D, D], F32, kind="ExternalInput").ap()
    projb_d = nc.dram_tensor("proj_b", [D], F32, kind="ExternalInput").ap()
    gamma_d = nc.dram_tensor("qn_gamma", [HD], F32, kind="ExternalInput").ap()
    beta_d = nc.dram_tensor("qn_beta", [HD], F32, kind="ExternalInput").ap()
    out_d = nc.dram_tensor("out", [N, D], F32, kind="ExternalOutput").ap()

    with tile.TileContext(nc) as tc:
        _emit(tc, out_d, x_d, qkvw_d, qkvb_d, projw_d, projb_d, gamma_d, beta_d,
              apply_gn)

    nc.compile()
    return nc


def _emit(tc, out_d, x_d, qkvw_d, qkvb_d, projw_d, projb_d, gamma_d, beta_d,
          apply_gn):
    nc = tc.nc
    ctx = ExitStack()
    with ctx:
        const = ctx.enter_context(tc.tile_pool(name="const", bufs=1))
        wpool = ctx.enter_context(tc.tile_pool(name="wts", bufs=1))
        data = ctx.enter_context(tc.tile_pool(name="data", bufs=1))
        epool = ctx.enter_context(tc.tile_pool(name="escore", bufs=2))
        qkpool = ctx.enter_context(tc.tile_pool(name="qk", bufs=2))
        tmpp = ctx.enter_context(tc.tile_pool(name="tmp", bufs=3))
        stat = ctx.enter_context(tc.tile_pool(name="stat", bufs=4))
        outp = ctx.enter_context(tc.tile_pool(name="outp", bufs=3))
        nrm = ctx.enter_context(tc.tile_pool(name="nrm", bufs=2))

        # ---- constants ----
        qkvb_bc = const.tile([P, O3], F32)
        nc.sync.dma_start(qkvb_bc[:], _bcast_ap(qkvb_d, P))
        projb_bc = const.tile([P, D], F32)
        nc.sync.dma_start(projb_bc[:], _bcast_ap(projb_d, P))
        eps_t = const.tile([P, 1], F32)
        nc.vector.memset(eps_t[:], EPS)
        ident = const.tile([P, P], BF16)
        make_identity(nc, ident[:])
        if apply_gn:
            gamma_bc = const.tile([P, HD], F32)
            nc.sync.dma_start(gamma_bc[:], _bcast_ap(gamma_d, P))
            beta_bc = const.tile([P, HD], F32)
            nc.sync.dma_start(beta_bc[:], _bcast_ap(beta_d, P))

        def pe_transpose_batch(pool, src_tiles, dst, dst_col0, evac_engine):
            """PE-transpose up to 4 [128,128] bf16 tiles through one PSUM bank;
            dst gets columns [dst_col0, dst_col0 + 128*len)."""
            ng = len(src_tiles)
            ps_full = pool.tile([P, 512], BF16, tag="tr", name="ps_tr_t")
            ps = ps_full[:, :ng * P]
            for i, src in enumerate(src_tiles):
                nc.tensor.transpose(ps_full[:, i * P:(i + 1) * P], src, ident[:])
            if evac_engine is nc.scalar:
                evac_engine.copy(dst[:, dst_col0:dst_col0 + ng * P], ps)
            else:
                evac_engine.tensor_copy(dst[:, dst_col0:dst_col0 + ng * P], ps)

        # ---- load + cast + PE-transpose x and weights into [k, ., m] layouts ----
        xT = wpool.tile([P, DC, N], BF16)      # [d_in, d_out, t]
        qkvwT = wpool.tile([P, DC, O3], BF16)  # [d_in, d_out, o]
        projwT = wpool.tile([P, DC, D], BF16)  # [o_in, o_out, e]

        # ---- phase 0+1 PSUM scope: transposes + qkv matmuls ----
        qn = data.tile([P, NT, D], BF16)            # [t_in, t_out, o]  (q heads)
        kn = data.tile([P, NT, D], BF16)
        # v with 64 ones-columns: attn@v psum rows 64:128 become the softmax
        # denominator s[i], broadcast across 64 partitions by the PE for free
        vext = data.tile([P, NT, H, 2 * HD], BF16)
        nc.vector.memset(vext[:, :, :, HD:2 * HD], 1.0)

        with tc.tile_pool(name="ps_tr", bufs=2, space="PSUM") as ps_tr, \
             tc.tile_pool(name="ps_mm", bufs=4, space="PSUM") as ps_mm, \
             tc.tile_pool(name="prep", bufs=2) as prep:

            def load_cast_transpose(src_d, n_rt, dstT):
                for gi, (g0, gn) in enumerate(_groups_of(n_rt, 4)):
                    stage = prep.tile([P, 4, D], BF16, tag="stage", name="stage")
                    for i in range(gn):
                        t_f = prep.tile([P, D], F32, tag="ld_f32", name="t_f")
                        nc.sync.dma_start(
                            t_f[:], src_d[(g0 + i) * P:(g0 + i + 1) * P, :]
                        )
                        nc.vector.tensor_copy(stage[:, i, :], t_f[:])
                    for dc in range(DC):
                        srcs = [stage[:, i, dc * P:(dc + 1) * P]
                                for i in range(gn)]
                        evac = nc.scalar if (gi + dc) % 2 == 0 else nc.vector
                        pe_transpose_batch(ps_tr, srcs, dstT[:, dc, :], g0 * P,
                                           evac)

            load_cast_transpose(x_d, NT, xT)
            load_cast_transpose(qkvw_d, O3 // P, qkvwT)

            # ---- QKV projection + bias + head-dim layernorm on q,k ----
            for tt in range(NT):
                for (c0, cs) in QKV_CHUNKS:
                    psum_full = ps_mm.tile([P, 512], F32, tag="mm", name="psum_mm")
                    psum = psum_full[:, :cs]
                    for dc in range(DC):
                        nc.tensor.matmul(
                            psum,
                            lhsT=xT[:, dc, tt * P:(tt + 1) * P],
                            rhs=qkvwT[:, dc, c0:c0 + cs],
                            start=(dc == 0),
                            stop=(dc == DC - 1),
                        )
                    if c0 < 2 * D:
                        # q/k chunk: bias add then LN over 64-wide segments
                        nsg = cs // HD
                        tmp_c_full = tmpp.tile([P, 512], F32, tag="tmpc", name="tmp_c")
                        tmp_c = tmp_c_full[:, :cs]
                        nc.vector.tensor_add(tmp_c, psum, qkvb_bc[:, c0:c0 + cs])
                        t3 = tmp_c.rearrange("p (s h) -> p s h", h=HD)
                        sums_full = stat.tile([P, 8], F32, tag="sums", name="sums")
                        sums = sums_full[:, :nsg]
                        nc.vector.tensor_reduce(
                            sums, t3, axis=mybir.AxisListType.X,
                            op=mybir.AluOpType.add
                        )
                        sq_full = tmpp.tile([P, 512], F32, tag="sq", name="sq")
                        sq = sq_full[:, :cs]
                        nc.scalar.square(sq, tmp_c)
                        sqs_full = stat.tile([P, 8], F32, tag="sqs", name="sqs")
                        sqs = sqs_full[:, :nsg]
                        nc.vector.tensor_reduce(
                            sqs,
                            sq.rearrange("p (s h) -> p s h", h=HD),
                            axis=mybir.AxisListType.X,
                            op=mybir.AluOpType.add,
                        )
                        mean_full = stat.tile([P, 8], F32, tag="mean", name="mean")
                        mean = mean_full[:, :nsg]
                        nc.vector.tensor_scalar_mul(mean, sums, 1.0 / HD)
                        msq_full = stat.tile([P, 8], F32, tag="msq", name="msq")
                        msq = msq_full[:, :nsg]
                        nc.vector.tensor_mul(msq, mean, mean)
                        var_full = stat.tile([P, 8], F32, tag="var", name="var")
                        var = var_full[:, :nsg]
                        nc.vector.tensor_scalar_mul(var, sqs, 1.0 / HD)
                        nc.vector.tensor_sub(var, var, msq)
                        std_full = stat.tile([P, 8], F32, tag="std", name="std")
                        std = std_full[:, :nsg]
                        nc.scalar.activation(
                            std, var, mybir.ActivationFunctionType.Sqrt,
                            bias=eps_t[:]
                        )
                        rstd_full = stat.tile([P, 8], F32, tag="rstd", name="rstd")
                        rstd = rstd_full[:, :nsg]
                        nc.vector.reciprocal(rstd, std)
                        # normalize: (tmp - mean) * rstd  (broadcast stats over HD)
                        mean_b = mean[:, :, None].to_broadcast((P, nsg, HD))
                        rstd_b = rstd[:, :, None].to_broadcast((P, nsg, HD))
                        nc.gpsimd.tensor_tensor(t3, t3, mean_b,
                                                op=mybir.AluOpType.subtract)
                        if apply_gn:
                            nc.gpsimd.tensor_tensor(t3, t3, rstd_b,
                                                    op=mybir.AluOpType.mult)
                            gamma_b = gamma_bc[:, None, :].to_broadcast((P, nsg, HD))
                            nc.gpsimd.tensor_tensor(t3, t3, gamma_b,
                                                    op=mybir.AluOpType.mult)
                        # write bf16 into qn/kn, splitting at q/k boundary (o=768)
                        spans = []
                        if c0 < D:
                            q_hi = min(c0 + cs, D)
                            spans.append((qn, c0, q_hi - c0, 0))
                        if c0 + cs > D:
                            k_lo = max(c0, D)
                            spans.append((kn, k_lo - D, c0 + cs - k_lo, k_lo - c0))
                        for (dst, d0, dlen, src_off) in spans:
                            nsg_s = dlen // HD
                            src = t3[:, src_off // HD:(src_off + dlen) // HD, :]
                            dgt = dst[:, tt, d0:d0 + dlen].rearrange(
                                "p (s h) -> p s h", h=HD
                            )
                            if apply_gn:
                                beta_b = beta_bc[:, None, :].to_broadcast(
                                    (P, nsg_s, HD))
                                nc.gpsimd.tensor_tensor(
                                    dgt, src, beta_b, op=mybir.AluOpType.add
                                )
                            else:
                                rstd_s = rstd_b[:, src_off // HD:
                                                (src_off + dlen) // HD, :]
                                nc.gpsimd.tensor_tensor(
                                    dgt, src, rstd_s, op=mybir.AluOpType.mult
                                )
                    else:
                        # v chunk: bias add, cast bf16, scatter into vext
                        hs = (c0 - 2 * D) // HD
                        nh = cs // HD
                        nc.vector.tensor_tensor(
                            vext[:, tt, hs:hs + nh, 0:HD],
                            psum.rearrange("p (s h) -> p s h", h=HD),
                            qkvb_bc[:, c0:c0 + cs].rearrange(
                                "p (s h) -> p s h", h=HD),
                            op=mybir.AluOpType.add,
                        )

            # proj_w prep fills the qkv phase's trailing gaps
            load_cast_transpose(projw_d, D // P, projwT)

        # ---- per-head attention ----
        # attnoutT [o_in, o_out, t] is written directly by the normalize step
        attnoutT = data.tile([P, DC, N], BF16)
        with tc.tile_pool(name="ps_tr2", bufs=2, space="PSUM") as ps_tr2, \
             tc.tile_pool(name="ps_st", bufs=2, space="PSUM") as ps_st, \
             tc.tile_pool(name="ps_av", bufs=2, space="PSUM") as ps_av:

            def emit_pair_transposes(hp):
                # qqT/kkT: [hd, t]; head 2hp in partitions 0:64, 2hp+1 in 64:128
                qqT = qkpool.tile([P, N], BF16, tag="qqT", name="qqT")
                kkT = qkpool.tile([P, N], BF16, tag="kkT", name="kkT")
                for (g0, gn) in _groups_of(NT, 4):
                    srcs_q = [qn[:, g0 + i, hp * P:(hp + 1) * P] for i in range(gn)]
                    pe_transpose_batch(ps_tr2, srcs_q, qqT, g0 * P, nc.vector)
                    srcs_k = [kn[:, g0 + i, hp * P:(hp + 1) * P] for i in range(gn)]
                    pe_transpose_batch(ps_tr2, srcs_k, kkT, g0 * P, nc.vector)
                return qqT, kkT

            def emit_scores(h, qqT, kkT):
                hh = h % 2
                qT = qqT[hh * HD:(hh + 1) * HD, :]
                kT = kkT[hh * HD:(hh + 1) * HD, :]
                # E[j, i] = exp(scale * k_j . q_i); one exp per j-tile
                E = epool.tile([P, NT, N], BF16, tag="E", name="E")
                for jt in range(NT):
                    ps = ps_st.tile([P, N], F32, tag="st", name="ps_st_t")
                    for ic in range(2):
                        nc.tensor.matmul(
                            ps[:, ic * 512:(ic + 1) * 512],
                            lhsT=kT[:, jt * P:(jt + 1) * P],
                            rhs=qT[:, ic * 512:(ic + 1) * 512],
                            start=True,
                            stop=True,
                        )
                    nc.scalar.activation(
                        E[:, jt, :],
                        ps,
                        mybir.ActivationFunctionType.Exp,
                        scale=SCALE,
                    )
                return E

            def emit_av(h, E):
                # psum[0:64, i]  = sum_j v[j, h'] E[j, i]
                # psum[64:128, i] = s[i] (denominator, PE-broadcast by the 64
                # ones-columns of vext); i-halves interleaved back-to-back
                pa0 = ps_av.tile([P, 512], F32, tag="av", name="pa0")
                pa1 = ps_av.tile([P, 512], F32, tag="av", name="pa1")
                for jt in range(NT):
                    nc.tensor.matmul(
                        pa0, lhsT=vext[:, jt, h, :], rhs=E[:, jt, 0:512],
                        start=(jt == 0), stop=(jt == NT - 1),
                    )
                    nc.tensor.matmul(
                        pa1, lhsT=vext[:, jt, h, :], rhs=E[:, jt, 512:1024],
                        start=(jt == 0), stop=(jt == NT - 1),
                    )
                for ic, pa in ((0, pa0), (1, pa1)):
                    rcp_t = nrm.tile([HD, 512], F32, tag="rcp_t", name="rcp_t")
                    nc.vector.reciprocal(rcp_t[:], pa[HD:2 * HD, :])
                    nc.vector.tensor_tensor(
                        attnoutT[(h % 2) * HD:(h % 2 + 1) * HD, h // 2,
                                 ic * 512:(ic + 1) * 512],
                        pa[0:HD, :],
                        rcp_t[:],
                        op=mybir.AluOpType.mult,
                    )

            # software pipeline: scores(h) emitted before av(h-1) so attn@v
            # matmuls fill TensorE gaps while ScalarE drains exps
            cur = emit_pair_transposes(0)
            prev = None  # (h, E)
            for h in range(H):
                hp, hh = divmod(h, 2)
                if hh == 0 and hp > 0:
                    cur = nxt
                qqT, kkT = cur
                E = emit_scores(h, qqT, kkT)
                if hh == 1 and hp + 1 < H // 2:
                    nxt = emit_pair_transposes(hp + 1)
                if prev is not None:
                    emit_av(*prev)
                prev = (h, E)
            emit_av(*prev)

        # ---- output projection ----
        EC = 384
        with tc.tile_pool(name="ps_pj", bufs=2, space="PSUM") as ps_pj:
            for tt in range(NT):
                for ec in range(D // EC):
                    ps_full = ps_pj.tile([P, 512], F32, tag="mm", name="ps_proj")
                    ps = ps_full[:, :EC]
                    for oc in range(DC):
                        nc.tensor.matmul(
                            ps,
                            lhsT=attnoutT[:, oc, tt * P:(tt + 1) * P],
                            rhs=projwT[:, oc, ec * EC:(ec + 1) * EC],
                            start=(oc == 0),
                            stop=(oc == DC - 1),
                        )
                    ot = outp.tile([P, EC], F32, tag="outt")
                    nc.vector.tensor_add(ot[:], ps,
                                         projb_bc[:, ec * EC:(ec + 1) * EC])
                    nc.sync.dma_start(
                        out_d[tt * P:(tt + 1) * P, ec * EC:(ec + 1) * EC], ot[:]
                    )


_NC_CACHE = {}


def _get_nc(apply_gn=True):
    if apply_gn not in _NC_CACHE:
        _NC_CACHE[apply_gn] = _build_graph(apply_gn)
    return _NC_CACHE[apply_gn]


def kernel(x, qkv_w, qkv_b, proj_w, proj_b, qn_gamma, qn_beta):
    qn_gamma = np.ascontiguousarray(qn_gamma, np.float32)
    qn_beta = np.ascontiguousarray(qn_beta, np.float32)
    apply_gn = not (np.all(qn_gamma == 1.0) and np.all(qn_beta == 0.0))
    nc = _get_nc(apply_gn)
    shared = {
        "qkv_w": np.ascontiguousarray(qkv_w, np.float32),
        "qkv_b": np.ascontiguousarray(qkv_b, np.float32),
        "proj_w": np.ascontiguousarray(proj_w, np.float32),
        "proj_b": np.ascontiguousarray(proj_b, np.float32),
        "qn_gamma": qn_gamma,
        "qn_beta": qn_beta,
    }
    in_maps = [
        {**shared, "x": np.ascontiguousarray(x[i], np.float32)} for i in range(B)
    ]
    res = run_bass_kernel_spmd(nc, in_maps, core_ids=list(range(B)))
    return np.stack([res.results[i]["out"] for i in range(B)], axis=0)


# revision 15
# speedup vs baseline: 1.1938x; 1.0107x over previous
"""Multi-head attention forward on 8 TRN2 NeuronCores, data-parallel over batch.

Reference computation (per batch element b):
    qkv  = x @ qkv_w.T + qkv_b                     # [N, 3D]
    q, k = LN_headdim(q), LN_headdim(k)            # layernorm over head_dim=64
    S    = q @ k.T * hd^-0.5 ; A = softmax_j(S)    # per head
    out  = (A @ v) @ proj_w.T + proj_b             # [N, D]

Kernel strategy (one batch element per core, no collectives):
  - bf16 matmuls on TensorE; f32 statistics/softmax denominators.
  - Scores computed TRANSPOSED: ST[j,i] = k_j . q_i so that E = exp(ST*scale)
    lands in SBUF with the contraction axis j on partitions -- E is directly
    the lhsT of the attn@v matmul (no attention-matrix transpose needed).
  - Softmax denominators come free: V gets a ones-column appended, so
    psum[i, 64] = sum_j E[j,i]; normalize with a per-partition scalar.
  - No max-subtraction in softmax: q,k are layernormed so |q.k|*scale <= 8,
    exp() is safely bounded (<= e^8) in f32/bf16.
  - All transposes on TensorE (identity matmul), batched 4 tiles into one
    [128,512] PSUM bank with a single evacuation copy.  DMA transposes are
    avoided entirely: they shatter into 256B packets (measured 780us of DMA
    engine time for this problem).
  - Engine balance: exp on ScalarE; reductions/psum-reads on VectorE;
    SBUF-only elementwise (casts, squares, LN scale) on GpSimd.
"""

import sys

import numpy as np

sys.path.insert(0, "/opt/trn_rl_repo")

from contextlib import ExitStack

import concourse.bass as bass
import concourse.tile as tile
from concourse import bacc, mybir
from concourse.bass_utils import run_bass_kernel_spmd
from concourse.masks import make_identity

B, N, D = 8, 1024, 768
H, HD = 12, 64
O3 = 3 * D  # 2304
P = 128
NT = N // P  # 8 token tiles
DC = D // P  # 6 contraction subtiles
EPS = 1e-5
SCALE = HD ** -0.5  # 0.125
F32 = mybir.dt.float32
BF16 = mybir.dt.bfloat16

# qkv output chunks: [start, size]; q = o[0:768), k = [768:1536), v = [1536:2304)
QKV_CHUNKS = [(0, 512), (512, 512), (1024, 512), (1536, 512), (2048, 256)]


def _bcast_ap(ap_1d, parts):
    """View a 1-D DRAM AP as [parts, n] with partition stride 0 (broadcast)."""
    return bass.AP(
        tensor=ap_1d.tensor,
        offset=ap_1d.offset,
        ap=[[0, parts]] + list(ap_1d.ap),
    )


def _groups_of(n, g):
    """Split range(n) into [(start, len)] groups of at most g."""
    return [(s, min(g, n - s)) for s in range(0, n, g)]


def _build_graph(apply_gn):
    nc = bacc.Bacc("TRN2", target_bir_lowering=False, debug=False, num_devices=B)

    x_d = nc.dram_tensor("x", [N, D], F32, kind="ExternalInput").ap()
    qkvw_d = nc.dram_tensor("qkv_w", [O3, D], F32, kind="ExternalInput").ap()
    qkvb_d = nc.dram_tensor("qkv_b", [O3], F32, kind="ExternalInput").ap()
    projw_d = nc.dram_tensor("proj_w", [D, D], F32, kind="ExternalInput").ap()
    projb_d = nc.dram_tensor("proj_b", [D], F32, kind="ExternalInput").ap()
    gamma_d = nc.dram_tensor("qn_gamma", [HD], F32, kind="ExternalInput").ap()
    beta_d = nc.dram_tensor("qn_beta", [HD], F32, kind="ExternalInput").ap()
    out_d = nc.dram_tensor("out", [N, D], F32, kind="ExternalOutput").ap()

    with tile.TileContext(nc) as tc:
        _emit(tc, out_d, x_d, qkvw_d, qkvb_d, projw_d, projb_d, gamma_d, beta_d,
              apply_gn)

    nc.compile()
    return nc


def _emit(tc, out_d, x_d, qkvw_d, qkvb_d, projw_d, projb_d, gamma_d, beta_d,
          apply_gn):
    nc = tc.nc
    ctx = ExitStack()
    with ctx:
        const = ctx.enter_context(tc.tile_pool(name="const", bufs=1))
        wpool = ctx.enter_context(tc.tile_pool(name="wts", bufs=1))
        data = ctx.enter_context(tc.tile_pool(name="data", bufs=1))
        epool = ctx.enter_context(tc.tile_pool(name="escore", bufs=2))
        qkpool = ctx.enter_context(tc.tile_pool(name="qk", bufs=2))
        tmpp = ctx.enter_context(tc.tile_pool(name="tmp", bufs=3))
        stat = ctx.enter_context(tc.tile_pool(name="stat", bufs=4))
        outp = ctx.enter_context(tc.tile_pool(name="outp", bufs=3))
        nrm = ctx.enter_context(tc.tile_pool(name="nrm", bufs=2))

        # ---- constants ----
        qkvb_bc = const.tile([P, O3], F32)
        nc.sync.dma_start(qkvb_bc[:], _bcast_ap(qkvb_d, P))
        projb_bc = const.tile([P, D], F32)
        nc.sync.dma_start(projb_bc[:], _bcast_ap(projb_d, P))
        eps_t = const.tile([P, 1], F32)
        nc.vector.memset(eps_t[:], EPS)
        ident = const.tile([P, P], BF16)
        make_identity(nc, ident[:])
        if apply_gn:
            gamma_bc = const.tile([P, HD], F32)
            nc.sync.dma_start(gamma_bc[:], _bcast_ap(gamma_d, P))
            beta_bc = const.tile([P, HD], F32)
            nc.sync.dma_start(beta_bc[:], _bcast_ap(beta_d, P))

        def pe_transpose_batch(pool, src_tiles, dst, dst_col0, evac_engine):
            """PE-transpose up to 4 [128,128] bf16 tiles through one PSUM bank;
            dst gets columns [dst_col0, dst_col0 + 128*len)."""
            ng = len(src_tiles)
            ps_full = pool.tile([P, 512], BF16, tag="tr", name="ps_tr_t")
            ps = ps_full[:, :ng * P]
            for i, src in enumerate(src_tiles):
                nc.tensor.transpose(ps_full[:, i * P:(i + 1) * P], src, ident[:])
            if evac_engine is nc.scalar:
                evac_engine.copy(dst[:, dst_col0:dst_col0 + ng * P], ps)
            else:
                evac_engine.tensor_copy(dst[:, dst_col0:dst_col0 + ng * P], ps)

        # ---- load + cast + PE-transpose x and weights into [k, ., m] layouts ----
        xT = wpool.tile([P, DC, N], BF16)      # [d_in, d_out, t]
        qkvwT = wpool.tile([P, DC, O3], BF16)  # [d_in, d_out, o]
        projwT = wpool.tile([P, DC, D], BF16)  # [o_in, o_out, e]

        # ---- phase 0+1 PSUM scope: transposes + qkv matmuls ----
        qn = data.tile([P, NT, D], BF16)            # [t_in, t_out, o]  (q heads)
        kn = data.tile([P, NT, D], BF16)
        # v with 64 ones-columns: attn@v psum rows 64:128 become the softmax
        # denominator s[i], broadcast across 64 partitions by the PE for free
        vext = data.tile([P, NT, H, 2 * HD], BF16)
        nc.vector.memset(vext[:, :, :, HD:2 * HD], 1.0)

        with tc.tile_pool(name="ps_tr", bufs=2, space="PSUM") as ps_tr, \
             tc.tile_pool(name="ps_mm", bufs=4, space="PSUM") as ps_mm, \
             tc.tile_pool(name="prep", bufs=2) as prep:

            def load_cast_transpose(src_d, n_rt, dstT):
                for gi, (g0, gn) in enumerate(_groups_of(n_rt, 4)):
                    stage = prep.tile([P, 4, D], BF16, tag="stage", name="stage")
                    for i in range(gn):
                        t_f = prep.tile([P, D], F32, tag="ld_f32", name="t_f")
                        nc.sync.dma_start(
                            t_f[:], src_d[(g0 + i) * P:(g0 + i + 1) * P, :]
                        )
                        nc.vector.tensor_copy(stage[:, i, :], t_f[:])
                    for dc in range(DC):
                        srcs = [stage[:, i, dc * P:(dc + 1) * P]
                                for i in range(gn)]
                        evac = nc.scalar if (gi + dc) % 2 == 0 else nc.vector
                        pe_transpose_batch(ps_tr, srcs, dstT[:, dc, :], g0 * P,
                                           evac)

            load_cast_transpose(x_d, NT, xT)
            load_cast_transpose(qkvw_d, O3 // P, qkvwT)

            # ---- QKV projection + bias + head-dim layernorm on q,k ----
            for tt in range(NT):
                for (c0, cs) in QKV_CHUNKS:
                    psum_full = ps_mm.tile([P, 512], F32, tag="mm", name="psum_mm")
                    psum = psum_full[:, :cs]
                    for dc in range(DC):
                        nc.tensor.matmul(
                            psum,
                            lhsT=xT[:, dc, tt * P:(tt + 1) * P],
                            rhs=qkvwT[:, dc, c0:c0 + cs],
                            start=(dc == 0),
                            stop=(dc == DC - 1),
                        )
                    if c0 < 2 * D:
                        # q/k chunk: bias add then LN over 64-wide segments
                        nsg = cs // HD
                        tmp_c_full = tmpp.tile([P, 512], F32, tag="tmpc", name="tmp_c")
                        tmp_c = tmp_c_full[:, :cs]
                        nc.vector.tensor_add(tmp_c, psum, qkvb_bc[:, c0:c0 + cs])
                        t3 = tmp_c.rearrange("p (s h) -> p s h", h=HD)
                        sums_full = stat.tile([P, 8], F32, tag="sums", name="sums")
                        sums = sums_full[:, :nsg]
                        nc.vector.tensor_reduce(
                            sums, t3, axis=mybir.AxisListType.X,
                            op=mybir.AluOpType.add
                        )
                        sq_full = tmpp.tile([P, 512], F32, tag="sq", name="sq")
                        sq = sq_full[:, :cs]
                        nc.scalar.square(sq, tmp_c)
                        sqs_full = stat.tile([P, 8], F32, tag="sqs", name="sqs")
                        sqs = sqs_full[:, :nsg]
                        nc.vector.tensor_reduce(
                            sqs,
                            sq.rearrange("p (s h) -> p s h", h=HD),
                            axis=mybir.AxisListType.X,
                            op=mybir.AluOpType.add,
                        )
                        mean_full = stat.tile([P, 8], F32, tag="mean", name="mean")
                        mean = mean_full[:, :nsg]
                        nc.vector.tensor_scalar_mul(mean, sums, 1.0 / HD)
                        msq_full = stat.tile([P, 8], F32, tag="msq", name="msq")
                        msq = msq_full[:, :nsg]
                        nc.vector.tensor_mul(msq, mean, mean)
                        var_full = stat.tile([P, 8], F32, tag="var", name="var")
                        var = var_full[:, :nsg]
                        nc.vector.tensor_scalar_mul(var, sqs, 1.0 / HD)
                        nc.vector.tensor_sub(var, var, msq)
                        std_full = stat.tile([P, 8], F32, tag="std", name="std")
                        std = std_full[:, :nsg]
                        nc.scalar.activation(
                            std, var, mybir.ActivationFunctionType.Sqrt,
                            bias=eps_t[:]
                        )
                        rstd_full = stat.tile([P, 8], F32, tag="rstd", name="rstd")
                        rstd = rstd_full[:, :nsg]
                        nc.vector.reciprocal(rstd, std)
                        # normalize: (tmp - mean) * rstd  (broadcast stats over HD)
                        mean_b = mean[:, :, None].to_broadcast((P, nsg, HD))
                        rstd_b = rstd[:, :, None].to_broadcast((P, nsg, HD))
                        nc.gpsimd.tensor_tensor(t3, t3, mean_b,
                                                op=mybir.AluOpType.subtract)
                        if apply_gn:
                            nc.gpsimd.tensor_tensor(t3, t3, rstd_b,
                                                    op=mybir.AluOpType.mult)
                            gamma_b = gamma_bc[:, None, :].to_broadcast((P, nsg, HD))
                            nc.gpsimd.tensor_tensor(t3, t3, gamma_b,
                                                    op=mybir.AluOpType.mult)
                        # write bf16 into qn/kn, splitting at q/k boundary (o=768)
                        spans = []
                        if c0 < D:
                            q_hi = min(c0 + cs, D)
                            spans.append((qn, c0, q_hi - c0, 0))
                        if c0 + cs > D:
                            k_lo = max(c0, D)
                            spans.append((kn, k_lo - D, c0 + cs - k_lo, k_lo - c0))
                        for (dst, d0, dlen, src_off) in spans:
                            nsg_s = dlen // HD
                            src = t3[:, src_off // HD:(src_off + dlen) // HD, :]
                            dgt = dst[:, tt, d0:d0 + dlen].rearrange(
                                "p (s h) -> p s h", h=HD
                            )
                            if apply_gn:
                                beta_b = beta_bc[:, None, :].to_broadcast(
                                    (P, nsg_s, HD))
                                nc.gpsimd.tensor_tensor(
                                    dgt, src, beta_b, op=mybir.AluOpType.add
                                )
                            else:
                                rstd_s = rstd_b[:, src_off // HD:
                                                (src_off + dlen) // HD, :]
                                nc.gpsimd.tensor_tensor(
                                    dgt, src, rstd_s, op=mybir.AluOpType.mult
                                )
                    else:
                        # v chunk: bias add, cast bf16, scatter into vext
                        hs = (c0 - 2 * D) // HD
                        nh = cs // HD
                        nc.vector.tensor_tensor(
                            vext[:, tt, hs:hs + nh, 0:HD],
                            psum.rearrange("p (s h) -> p s h", h=HD),
                            qkvb_bc[:, c0:c0 + cs].rearrange(
                                "p (s h) -> p s h", h=HD),
                            op=mybir.AluOpType.add,
                        )

            # proj_w prep fills the qkv phase's trailing gaps
            load_cast_transpose(projw_d, D // P, projwT)

        # ---- per-head attention ----
        # attnoutT [o_in, o_out, t] is written directly by the normalize step
        attnoutT = data.tile([P, DC, N], BF16)
        with tc.tile_pool(name="ps_tr2", bufs=1, space="PSUM") as ps_tr2, \
             tc.tile_pool(name="ps_st", bufs=3, space="PSUM") as ps_st, \
             tc.tile_pool(name="ps_av", bufs=4, space="PSUM") as ps_av:

            def emit_pair_transposes(hp):
                # qqT/kkT: [hd, t]; head 2hp in partitions 0:64, 2hp+1 in 64:128
                qqT = qkpool.tile([P, N], BF16, tag="qqT", name="qqT")
                kkT = qkpool.tile([P, N], BF16, tag="kkT", name="kkT")
                for (g0, gn) in _groups_of(NT, 4):
                    srcs_q = [qn[:, g0 + i, hp * P:(hp + 1) * P] for i in range(gn)]
                    pe_transpose_batch(ps_tr2, srcs_q, qqT, g0 * P, nc.vector)
                    srcs_k = [kn[:, g0 + i, hp * P:(hp + 1) * P] for i in range(gn)]
                    pe_transpose_batch(ps_tr2, srcs_k, kkT, g0 * P, nc.vector)
                return qqT, kkT

            def emit_scores(h, qqT, kkT):
                hh = h % 2
                qT = qqT[hh * HD:(hh + 1) * HD, :]
                kT = kkT[hh * HD:(hh + 1) * HD, :]
                # E[j, i] = exp(scale * k_j . q_i); one exp per j-tile
                E = epool.tile([P, NT, N], BF16, tag="E", name="E")
                for jt in range(NT):
                    for ic in range(2):
                        ps = ps_st.tile([P, 512], F32, tag="st", name="ps_st_t")
                        nc.tensor.matmul(
                            ps,
                            lhsT=kT[:, jt * P:(jt + 1) * P],
                            rhs=qT[:, ic * 512:(ic + 1) * 512],
                            start=True,
                            stop=True,
                        )
                        nc.scalar.activation(
                            E[:, jt, ic * 512:(ic + 1) * 512],
                            ps,
                            mybir.ActivationFunctionType.Exp,
                            scale=SCALE,
                        )
                return E

            def emit_av(h, E):
                # psum[0:64, i]  = sum_j v[j, h'] E[j, i]
                # psum[64:128, i] = s[i] (denominator, PE-broadcast by the 64
                # ones-columns of vext); i-halves interleaved back-to-back
                pa0 = ps_av.tile([P, 512], F32, tag="av", name="pa0")
                pa1 = ps_av.tile([P, 512], F32, tag="av", name="pa1")
                for jt in range(NT):
                    nc.tensor.matmul(
                        pa0, lhsT=vext[:, jt, h, :], rhs=E[:, jt, 0:512],
                        start=(jt == 0), stop=(jt == NT - 1),
                    )
                    nc.tensor.matmul(
                        pa1, lhsT=vext[:, jt, h, :], rhs=E[:, jt, 512:1024],
                        start=(jt == 0), stop=(jt == NT - 1),
                    )
                for ic, pa in ((0, pa0), (1, pa1)):
                    s_sb = nrm.tile([HD, 512], F32, tag="s_sb", name="s_sb")
                    nc.vector.tensor_copy(s_sb[:], pa[HD:2 * HD, :])
                    rcp_t = nrm.tile([HD, 512], F32, tag="rcp_t", name="rcp_t")
                    nc.vector.reciprocal_approx_fast(rcp_t[:], s_sb[:])
                    nc.vector.tensor_tensor(
                        attnoutT[(h % 2) * HD:(h % 2 + 1) * HD, h // 2,
                                 ic * 512:(ic + 1) * 512],
                        pa[0:HD, :],
                        rcp_t[:],
                        op=mybir.AluOpType.mult,
                    )

            # software pipeline: scores(h) emitted before av(h-1) so attn@v
            # matmuls fill TensorE gaps while ScalarE drains exps
            cur = emit_pair_transposes(0)
            prev = None  # (h, E)
            for h in range(H):
                hp, hh = divmod(h, 2)
                if hh == 0 and hp > 0:
                    cur = nxt
                qqT, kkT = cur
                E = emit_scores(h, qqT, kkT)
                if hh == 1 and hp + 1 < H // 2:
                    nxt = emit_pair_transposes(hp + 1)
                if prev is not None:
                    emit_av(*prev)
                prev = (h, E)
            emit_av(*prev)

        # ---- output projection ----
        EC = 384
        with tc.tile_pool(name="ps_pj", bufs=2, space="PSUM") as ps_pj:
            for tt in range(NT):
                for ec in range(D // EC):
                    ps_full = ps_pj.tile([P, 512], F32, tag="mm", name="ps_proj")
                    ps = ps_full[:, :EC]
                    for oc in range(DC):
                        nc.tensor.matmul(
                            ps,
                            lhsT=attnoutT[:, oc, tt * P:(tt + 1) * P],
                            rhs=projwT[:, oc, ec * EC:(ec + 1) * EC],
                            start=(oc == 0),
                            stop=(oc == DC - 1),
                        )
                    ot = outp.tile([P, EC], F32, tag="outt")
                    nc.vector.tensor_add(ot[:], ps,
                                         projb_bc[:, ec * EC:(ec + 1) * EC])
                    nc.sync.dma_start(
                        out_d[tt * P:(tt + 1) * P, ec * EC:(ec + 1) * EC], ot[:]
                    )


_NC_CACHE = {}


def _get_nc(apply_gn=True):
    if apply_gn not in _NC_CACHE:
        _NC_CACHE[apply_gn] = _build_graph(apply_gn)
    return _NC_CACHE[apply_gn]


def kernel(x, qkv_w, qkv_b, proj_w, proj_b, qn_gamma, qn_beta):
    qn_gamma = np.ascontiguousarray(qn_gamma, np.float32)
    qn_beta = np.ascontiguousarray(qn_beta, np.float32)
    apply_gn = not (np.all(qn_gamma == 1.0) and np.all(qn_beta == 0.0))
    nc = _get_nc(apply_gn)
    shared = {
        "qkv_w": np.ascontiguousarray(qkv_w, np.float32),
        "qkv_b": np.ascontiguousarray(qkv_b, np.float32),
        "proj_w": np.ascontiguousarray(proj_w, np.float32),
        "proj_b": np.ascontiguousarray(proj_b, np.float32),
        "qn_gamma": qn_gamma,
        "qn_beta": qn_beta,
    }
    in_maps = [
        {**shared, "x": np.ascontiguousarray(x[i], np.float32)} for i in range(B)
    ]
    res = run_bass_kernel_spmd(nc, in_maps, core_ids=list(range(B)))
    return np.stack([res.results[i]["out"] for i in range(B)], axis=0)


# revision 16
# speedup vs baseline: 1.2192x; 1.0213x over previous
"""Multi-head attention forward on 8 TRN2 NeuronCores, data-parallel over batch.

Reference computation (per batch element b):
    qkv  = x @ qkv_w.T + qkv_b                     # [N, 3D]
    q, k = LN_headdim(q), LN_headdim(k)            # layernorm over head_dim=64
    S    = q @ k.T * hd^-0.5 ; A = softmax_j(S)    # per head
    out  = (A @ v) @ proj_w.T + proj_b             # [N, D]

Kernel strategy (one batch element per core, no collectives):
  - bf16 matmuls on TensorE; f32 statistics/softmax denominators.
  - Scores computed TRANSPOSED: ST[j,i] = k_j . q_i so that E = exp(ST*scale)
    lands in SBUF with the contraction axis j on partitions -- E is directly
    the lhsT of the attn@v matmul (no attention-matrix transpose needed).
  - Softmax denominators come free: V gets a ones-column appended, so
    psum[i, 64] = sum_j E[j,i]; normalize with a per-partition scalar.
  - No max-subtraction in softmax: q,k are layernormed so |q.k|*scale <= 8,
    exp() is safely bounded (<= e^8) in f32/bf16.
  - All transposes on TensorE (identity matmul), batched 4 tiles into one
    [128,512] PSUM bank with a single evacuation copy.  DMA transposes are
    avoided entirely: they shatter into 256B packets (measured 780us of DMA
    engine time for this problem).
  - Engine balance: exp on ScalarE; reductions/psum-reads on VectorE;
    SBUF-only elementwise (casts, squares, LN scale) on GpSimd.
"""

import sys

import numpy as np

sys.path.insert(0, "/opt/trn_rl_repo")

from contextlib import ExitStack

import concourse.bass as bass
import concourse.tile as tile
from concourse import bacc, mybir
from concourse.bass_utils import run_bass_kernel_spmd
from concourse.masks import make_identity

B, N, D = 8, 1024, 768
H, HD = 12, 64
O3 = 3 * D  # 2304
P = 128
NT = N // P  # 8 token tiles
DC = D // P  # 6 contraction subtiles
EPS = 1e-5
SCALE = HD ** -0.5  # 0.125
F32 = mybir.dt.float32
BF16 = mybir.dt.bfloat16

# qkv output chunks: [start, size]; q = o[0:768), k = [768:1536), v = [1536:2304)
QKV_CHUNKS = [(0, 512), (512, 512), (1024, 512), (1536, 512), (2048, 256)]


def _bcast_ap(ap_1d, parts):
    """View a 1-D DRAM AP as [parts, n] with partition stride 0 (broadcast)."""
    return bass.AP(
        tensor=ap_1d.tensor,
        offset=ap_1d.offset,
        ap=[[0, parts]] + list(ap_1d.ap),
    )


def _groups_of(n, g):
    """Split range(n) into [(start, len)] groups of at most g."""
    return [(s, min(g, n - s)) for s in range(0, n, g)]


def _build_graph(apply_gn):
    nc = bacc.Bacc("TRN2", target_bir_lowering=False, debug=False, num_devices=B)

    x_d = nc.dram_tensor("x", [N, D], F32, kind="ExternalInput").ap()
    qkvw_d = nc.dram_tensor("qkv_w", [O3, D], F32, kind="ExternalInput").ap()
    qkvb_d = nc.dram_tensor("qkv_b", [O3], F32, kind="ExternalInput").ap()
    projw_d = nc.dram_tensor("proj_w", [D, D], F32, kind="ExternalInput").ap()
    projb_d = nc.dram_tensor("proj_b", [D], F32, kind="ExternalInput").ap()
    gamma_d = nc.dram_tensor("qn_gamma", [HD], F32, kind="ExternalInput").ap()
    beta_d = nc.dram_tensor("qn_beta", [HD], F32, kind="ExternalInput").ap()
    out_d = nc.dram_tensor("out", [N, D], F32, kind="ExternalOutput").ap()

    with tile.TileContext(nc) as tc:
        _emit(tc, out_d, x_d, qkvw_d, qkvb_d, projw_d, projb_d, gamma_d, beta_d,
              apply_gn)

    nc.compile()
    return nc


def _emit(tc, out_d, x_d, qkvw_d, qkvb_d, projw_d, projb_d, gamma_d, beta_d,
          apply_gn):
    nc = tc.nc
    ctx = ExitStack()
    with ctx:
        const = ctx.enter_context(tc.tile_pool(name="const", bufs=1))
        wpool = ctx.enter_context(tc.tile_pool(name="wts", bufs=1))
        data = ctx.enter_context(tc.tile_pool(name="data", bufs=1))
        epool = ctx.enter_context(tc.tile_pool(name="escore", bufs=2))
        qkpool = ctx.enter_context(tc.tile_pool(name="qk", bufs=2))
        tmpp = ctx.enter_context(tc.tile_pool(name="tmp", bufs=3))
        stat = ctx.enter_context(tc.tile_pool(name="stat", bufs=4))
        outp = ctx.enter_context(tc.tile_pool(name="outp", bufs=3))
        nrm = ctx.enter_context(tc.tile_pool(name="nrm", bufs=2))

        # ---- constants ----
        qkvb_bc = const.tile([P, O3], F32)
        nc.sync.dma_start(qkvb_bc[:], _bcast_ap(qkvb_d, P))
        projb_bc = const.tile([P, D], F32)
        nc.sync.dma_start(projb_bc[:], _bcast_ap(projb_d, P))
        eps_t = const.tile([P, 1], F32)
        nc.vector.memset(eps_t[:], EPS)
        ident = const.tile([P, P], BF16)
        make_identity(nc, ident[:])
        if apply_gn:
            gamma_bc = const.tile([P, HD], F32)
            nc.sync.dma_start(gamma_bc[:], _bcast_ap(gamma_d, P))
            beta_bc = const.tile([P, HD], F32)
            nc.sync.dma_start(beta_bc[:], _bcast_ap(beta_d, P))

        def pe_transpose_batch(pool, src_tiles, dst, dst_col0, evac_engine):
            """PE-transpose up to 4 [128,128] bf16 tiles through one PSUM bank;
            dst gets columns [dst_col0, dst_col0 + 128*len)."""
            ng = len(src_tiles)
            ps_full = pool.tile([P, 512], BF16, tag="tr", name="ps_tr_t")
            ps = ps_full[:, :ng * P]
            for i, src in enumerate(src_tiles):
                nc.tensor.transpose(ps_full[:, i * P:(i + 1) * P], src, ident[:])
            if evac_engine is nc.scalar:
                evac_engine.copy(dst[:, dst_col0:dst_col0 + ng * P], ps)
            else:
                evac_engine.tensor_copy(dst[:, dst_col0:dst_col0 + ng * P], ps)

        # ---- load + cast + PE-transpose x and weights into [k, ., m] layouts ----
        xT = wpool.tile([P, DC, N], BF16)      # [d_in, d_out, t]
        qkvwT = wpool.tile([P, DC, O3], BF16)  # [d_in, d_out, o]
        projwT = wpool.tile([P, DC, D], BF16)  # [o_in, o_out, e]

        # ---- phase 0+1 PSUM scope: transposes + qkv matmuls ----
        qn = data.tile([P, NT, D], BF16)            # [t_in, t_out, o]  (q heads)
        kn = data.tile([P, NT, D], BF16)
        # v with 64 ones-columns: attn@v psum rows 64:128 become the softmax
        # denominator s[i], broadcast across 64 partitions by the PE for free
        vext = data.tile([P, NT, H, 2 * HD], BF16)
        nc.vector.memset(vext[:, :, :, HD:2 * HD], 1.0)

        with tc.tile_pool(name="ps_tr", bufs=2, space="PSUM") as ps_tr, \
             tc.tile_pool(name="ps_mm", bufs=4, space="PSUM") as ps_mm, \
             tc.tile_pool(name="prep", bufs=2) as prep:

            def load_cast_transpose(src_d, n_rt, dstT):
                for gi, (g0, gn) in enumerate(_groups_of(n_rt, 4)):
                    stage = prep.tile([P, 4, D], BF16, tag="stage", name="stage")
                    for i in range(gn):
                        t_f = prep.tile([P, D], F32, tag="ld_f32", name="t_f")
                        nc.sync.dma_start(
                            t_f[:], src_d[(g0 + i) * P:(g0 + i + 1) * P, :]
                        )
                        nc.vector.tensor_copy(stage[:, i, :], t_f[:])
                    for dc in range(DC):
                        srcs = [stage[:, i, dc * P:(dc + 1) * P]
                                for i in range(gn)]
                        evac = nc.scalar if (gi + dc) % 2 == 0 else nc.vector
                        pe_transpose_batch(ps_tr, srcs, dstT[:, dc, :], g0 * P,
                                           evac)

            load_cast_transpose(x_d, NT, xT)
            load_cast_transpose(qkvw_d, O3 // P, qkvwT)

            # ---- QKV projection + bias + head-dim layernorm on q,k ----
            for tt in range(NT):
                for (c0, cs) in QKV_CHUNKS:
                    psum_full = ps_mm.tile([P, 512], F32, tag="mm", name="psum_mm")
                    psum = psum_full[:, :cs]
                    for dc in range(DC):
                        nc.tensor.matmul(
                            psum,
                            lhsT=xT[:, dc, tt * P:(tt + 1) * P],
                            rhs=qkvwT[:, dc, c0:c0 + cs],
                            start=(dc == 0),
                            stop=(dc == DC - 1),
                        )
                    if c0 < 2 * D:
                        # q/k chunk: bias add then LN over 64-wide segments
                        nsg = cs // HD
                        tmp_c_full = tmpp.tile([P, 512], F32, tag="tmpc", name="tmp_c")
                        tmp_c = tmp_c_full[:, :cs]
                        nc.vector.tensor_add(tmp_c, psum, qkvb_bc[:, c0:c0 + cs])
                        t3 = tmp_c.rearrange("p (s h) -> p s h", h=HD)
                        sums_full = stat.tile([P, 8], F32, tag="sums", name="sums")
                        sums = sums_full[:, :nsg]
                        nc.vector.tensor_reduce(
                            sums, t3, axis=mybir.AxisListType.X,
                            op=mybir.AluOpType.add
                        )
                        sq_full = tmpp.tile([P, 512], F32, tag="sq", name="sq")
                        sq = sq_full[:, :cs]
                        nc.scalar.square(sq, tmp_c)
                        sqs_full = stat.tile([P, 8], F32, tag="sqs", name="sqs")
                        sqs = sqs_full[:, :nsg]
                        nc.vector.tensor_reduce(
                            sqs,
                            sq.rearrange("p (s h) -> p s h", h=HD),
                            axis=mybir.AxisListType.X,
                            op=mybir.AluOpType.add,
                        )
                        mean_full = stat.tile([P, 8], F32, tag="mean", name="mean")
                        mean = mean_full[:, :nsg]
                        nc.vector.tensor_scalar_mul(mean, sums, 1.0 / HD)
                        msq_full = stat.tile([P, 8], F32, tag="msq", name="msq")
                        msq = msq_full[:, :nsg]
                        nc.vector.tensor_mul(msq, mean, mean)
                        var_full = stat.tile([P, 8], F32, tag="var", name="var")
                        var = var_full[:, :nsg]
                        nc.vector.tensor_scalar_mul(var, sqs, 1.0 / HD)
                        nc.vector.tensor_sub(var, var, msq)
                        std_full = stat.tile([P, 8], F32, tag="std", name="std")
                        std = std_full[:, :nsg]
                        nc.scalar.activation(
                            std, var, mybir.ActivationFunctionType.Sqrt,
                            bias=eps_t[:]
                        )
                        rstd_full = stat.tile([P, 8], F32, tag="rstd", name="rstd")
                        rstd = rstd_full[:, :nsg]
                        nc.vector.reciprocal(rstd, std)
                        # normalize: (tmp - mean) * rstd  (broadcast stats over HD)
                        mean_b = mean[:, :, None].to_broadcast((P, nsg, HD))
                        rstd_b = rstd[:, :, None].to_broadcast((P, nsg, HD))
                        nc.gpsimd.tensor_tensor(t3, t3, mean_b,
                                                op=mybir.AluOpType.subtract)
                        if apply_gn:
                            nc.gpsimd.tensor_tensor(t3, t3, rstd_b,
                                                    op=mybir.AluOpType.mult)
                            gamma_b = gamma_bc[:, None, :].to_broadcast((P, nsg, HD))
                            nc.gpsimd.tensor_tensor(t3, t3, gamma_b,
                                                    op=mybir.AluOpType.mult)
                        # write bf16 into qn/kn, splitting at q/k boundary (o=768)
                        spans = []
                        if c0 < D:
                            q_hi = min(c0 + cs, D)
                            spans.append((qn, c0, q_hi - c0, 0))
                        if c0 + cs > D:
                            k_lo = max(c0, D)
                            spans.append((kn, k_lo - D, c0 + cs - k_lo, k_lo - c0))
                        for (dst, d0, dlen, src_off) in spans:
                            nsg_s = dlen // HD
                            src = t3[:, src_off // HD:(src_off + dlen) // HD, :]
                            dgt = dst[:, tt, d0:d0 + dlen].rearrange(
                                "p (s h) -> p s h", h=HD
                            )
                            if apply_gn:
                                beta_b = beta_bc[:, None, :].to_broadcast(
                                    (P, nsg_s, HD))
                                nc.gpsimd.tensor_tensor(
                                    dgt, src, beta_b, op=mybir.AluOpType.add
                                )
                            else:
                                rstd_s = rstd_b[:, src_off // HD:
                                                (src_off + dlen) // HD, :]
                                nc.gpsimd.tensor_tensor(
                                    dgt, src, rstd_s, op=mybir.AluOpType.mult
                                )
                    else:
                        # v chunk: bias add, cast bf16, scatter into vext
                        hs = (c0 - 2 * D) // HD
                        nh = cs // HD
                        nc.vector.tensor_tensor(
                            vext[:, tt, hs:hs + nh, 0:HD],
                            psum.rearrange("p (s h) -> p s h", h=HD),
                            qkvb_bc[:, c0:c0 + cs].rearrange(
                                "p (s h) -> p s h", h=HD),
                            op=mybir.AluOpType.add,
                        )

            # proj_w prep fills the qkv phase's trailing gaps
            load_cast_transpose(projw_d, D // P, projwT)

        # ---- per-head attention ----
        # attnoutT [o_in, o_out, t] is written directly by the normalize step
        attnoutT = data.tile([P, DC, N], BF16)
        with tc.tile_pool(name="ps_tr2", bufs=1, space="PSUM") as ps_tr2, \
             tc.tile_pool(name="ps_st", bufs=3, space="PSUM") as ps_st, \
             tc.tile_pool(name="ps_av", bufs=4, space="PSUM") as ps_av:

            def emit_pair_transposes(hp):
                # qqT/kkT: [hd, t]; head 2hp in partitions 0:64, 2hp+1 in 64:128
                qqT = qkpool.tile([P, N], BF16, tag="qqT", name="qqT")
                kkT = qkpool.tile([P, N], BF16, tag="kkT", name="kkT")
                for (g0, gn) in _groups_of(NT, 4):
                    srcs_q = [qn[:, g0 + i, hp * P:(hp + 1) * P] for i in range(gn)]
                    pe_transpose_batch(ps_tr2, srcs_q, qqT, g0 * P, nc.vector)
                    srcs_k = [kn[:, g0 + i, hp * P:(hp + 1) * P] for i in range(gn)]
                    pe_transpose_batch(ps_tr2, srcs_k, kkT, g0 * P, nc.vector)
                return qqT, kkT

            def emit_head(h, qqT, kkT, prev):
                """Emit scores+exp for head h, 1:1 interleaved with the
                attn@v accumulation of head h-1 (prev) so the static TensorE
                stream has ready work during every exp drain."""
                hh = h % 2
                qT = qqT[hh * HD:(hh + 1) * HD, :]
                kT = kkT[hh * HD:(hh + 1) * HD, :]
                E = epool.tile([P, NT, N], BF16, tag="E", name="E")
                if prev is not None:
                    hprev, Eprev = prev
                    pa0 = ps_av.tile([P, 512], F32, tag="av", name="pa0")
                    pa1 = ps_av.tile([P, 512], F32, tag="av", name="pa1")
                for jt in range(NT):
                    for ic in range(2):
                        ps = ps_st.tile([P, 512], F32, tag="st", name="ps_st_t")
                        nc.tensor.matmul(
                            ps,
                            lhsT=kT[:, jt * P:(jt + 1) * P],
                            rhs=qT[:, ic * 512:(ic + 1) * 512],
                            start=True,
                            stop=True,
                        )
                        nc.scalar.activation(
                            E[:, jt, ic * 512:(ic + 1) * 512],
                            ps,
                            mybir.ActivationFunctionType.Exp,
                            scale=SCALE,
                        )
                    if prev is not None:
                        nc.tensor.matmul(
                            pa0, lhsT=vext[:, jt, hprev, :],
                            rhs=Eprev[:, jt, 0:512],
                            start=(jt == 0), stop=(jt == NT - 1),
                        )
                        nc.tensor.matmul(
                            pa1, lhsT=vext[:, jt, hprev, :],
                            rhs=Eprev[:, jt, 512:1024],
                            start=(jt == 0), stop=(jt == NT - 1),
                        )
                if prev is not None:
                    emit_normalize(hprev, pa0, pa1)
                return E

            def emit_av_tail(h, E):
                pa0 = ps_av.tile([P, 512], F32, tag="av", name="pa0")
                pa1 = ps_av.tile([P, 512], F32, tag="av", name="pa1")
                for jt in range(NT):
                    nc.tensor.matmul(
                        pa0, lhsT=vext[:, jt, h, :], rhs=E[:, jt, 0:512],
                        start=(jt == 0), stop=(jt == NT - 1),
                    )
                    nc.tensor.matmul(
                        pa1, lhsT=vext[:, jt, h, :], rhs=E[:, jt, 512:1024],
                        start=(jt == 0), stop=(jt == NT - 1),
                    )
                emit_normalize(h, pa0, pa1)

            def emit_normalize(h, pa0, pa1):
                for ic, pa in ((0, pa0), (1, pa1)):
                    s_sb = nrm.tile([HD, 512], F32, tag="s_sb", name="s_sb")
                    nc.vector.tensor_copy(s_sb[:], pa[HD:2 * HD, :])
                    rcp_t = nrm.tile([HD, 512], F32, tag="rcp_t", name="rcp_t")
                    nc.vector.reciprocal_approx_fast(rcp_t[:], s_sb[:])
                    nc.vector.tensor_tensor(
                        attnoutT[(h % 2) * HD:(h % 2 + 1) * HD, h // 2,
                                 ic * 512:(ic + 1) * 512],
                        pa[0:HD, :],
                        rcp_t[:],
                        op=mybir.AluOpType.mult,
                    )

            cur = emit_pair_transposes(0)
            prev = None  # (h, E)
            for h in range(H):
                hp, hh = divmod(h, 2)
                if hh == 0 and hp > 0:
                    cur = nxt
                qqT, kkT = cur
                E = emit_head(h, *cur, prev)
                if hh == 1 and hp + 1 < H // 2:
                    nxt = emit_pair_transposes(hp + 1)
                prev = (h, E)
            emit_av_tail(*prev)

        # ---- output projection ----
        EC = 384
        with tc.tile_pool(name="ps_pj", bufs=2, space="PSUM") as ps_pj:
            for tt in range(NT):
                for ec in range(D // EC):
                    ps_full = ps_pj.tile([P, 512], F32, tag="mm", name="ps_proj")
                    ps = ps_full[:, :EC]
                    for oc in range(DC):
                        nc.tensor.matmul(
                            ps,
                            lhsT=attnoutT[:, oc, tt * P:(tt + 1) * P],
                            rhs=projwT[:, oc, ec * EC:(ec + 1) * EC],
                            start=(oc == 0),
                            stop=(oc == DC - 1),
                        )
                    ot = outp.tile([P, EC], F32, tag="outt")
                    nc.vector.tensor_add(ot[:], ps,
                                         projb_bc[:, ec * EC:(ec + 1) * EC])
                    nc.sync.dma_start(
                        out_d[tt * P:(tt + 1) * P, ec * EC:(ec + 1) * EC], ot[:]
                    )


_NC_CACHE = {}


def _get_nc(apply_gn=True):
    if apply_gn not in _NC_CACHE:
        _NC_CACHE[apply_gn] = _build_graph(apply_gn)
    return _NC_CACHE[apply_gn]


def kernel(x, qkv_w, qkv_b, proj_w, proj_b, qn_gamma, qn_beta):
    qn_gamma = np.ascontiguousarray(qn_gamma, np.float32)
    qn_beta = np.ascontiguousarray(qn_beta, np.float32)
    apply_gn = not (np.all(qn_gamma == 1.0) and np.all(qn_beta == 0.0))
    nc = _get_nc(apply_gn)
    shared = {
        "qkv_w": np.ascontiguousarray(qkv_w, np.float32),
        "qkv_b": np.ascontiguousarray(qkv_b, np.float32),
        "proj_w": np.ascontiguousarray(proj_w, np.float32),
        "proj_b": np.ascontiguousarray(proj_b, np.float32),
        "qn_gamma": qn_gamma,
        "qn_beta": qn_beta,
    }
    in_maps = [
        {**shared, "x": np.ascontiguousarray(x[i], np.float32)} for i in range(B)
    ]
    res = run_bass_kernel_spmd(nc, in_maps, core_ids=list(range(B)))
    return np.stack([res.results[i]["out"] for i in range(B)], axis=0)


# revision 17
# speedup vs baseline: 1.3987x; 1.1473x over previous
"""Multi-head attention forward on 8 TRN2 NeuronCores, data-parallel over batch.

Reference computation (per batch element b):
    qkv  = x @ qkv_w.T + qkv_b                     # [N, 3D]
    q, k = LN_headdim(q), LN_headdim(k)            # layernorm over head_dim=64
    S    = q @ k.T * hd^-0.5 ; A = softmax_j(S)    # per head
    out  = (A @ v) @ proj_w.T + proj_b             # [N, D]

Kernel strategy (one batch element per core, no collectives):
  - bf16 matmuls on TensorE; f32 statistics/softmax denominators.
  - Scores computed TRANSPOSED: ST[j,i] = k_j . q_i so that E = exp(ST*scale)
    lands in SBUF with the contraction axis j on partitions -- E is directly
    the lhsT of the attn@v matmul (no attention-matrix transpose needed).
  - Softmax denominators come free: V gets a ones-column appended, so
    psum[i, 64] = sum_j E[j,i]; normalize with a per-partition scalar.
  - No max-subtraction in softmax: q,k are layernormed so |q.k|*scale <= 8,
    exp() is safely bounded (<= e^8) in f32/bf16.
  - All transposes on TensorE (identity matmul), batched 4 tiles into one
    [128,512] PSUM bank with a single evacuation copy.  DMA transposes are
    avoided entirely: they shatter into 256B packets (measured 780us of DMA
    engine time for this problem).
  - Engine balance: exp on ScalarE; reductions/psum-reads on VectorE;
    SBUF-only elementwise (casts, squares, LN scale) on GpSimd.
"""

import sys

import numpy as np

sys.path.insert(0, "/opt/trn_rl_repo")

from contextlib import ExitStack

import concourse.bass as bass
import concourse.tile as tile
from concourse import bacc, mybir
from concourse.bass_utils import run_bass_kernel_spmd
from concourse.masks import make_identity

B, N, D = 8, 1024, 768
H, HD = 12, 64
O3 = 3 * D  # 2304
P = 128
NT = N // P  # 8 token tiles
DC = D // P  # 6 contraction subtiles
EPS = 1e-5
SCALE = HD ** -0.5  # 0.125
F32 = mybir.dt.float32
BF16 = mybir.dt.bfloat16

# qkv output chunks: [start, size]; q = o[0:768), k = [768:1536), v = [1536:2304)
QKV_CHUNKS = [(0, 512), (512, 512), (1024, 512), (1536, 512), (2048, 256)]


def _bcast_ap(ap_1d, parts):
    """View a 1-D DRAM AP as [parts, n] with partition stride 0 (broadcast)."""
    return bass.AP(
        tensor=ap_1d.tensor,
        offset=ap_1d.offset,
        ap=[[0, parts]] + list(ap_1d.ap),
    )


def _groups_of(n, g):
    """Split range(n) into [(start, len)] groups of at most g."""
    return [(s, min(g, n - s)) for s in range(0, n, g)]


def _build_graph(apply_gn):
    nc = bacc.Bacc("TRN2", target_bir_lowering=False, debug=False, num_devices=B)

    x_d = nc.dram_tensor("x", [N, D], F32, kind="ExternalInput").ap()
    qkvw_d = nc.dram_tensor("qkv_w", [O3, D], F32, kind="ExternalInput").ap()
    qkvb_d = nc.dram_tensor("qkv_b", [O3], F32, kind="ExternalInput").ap()
    projw_d = nc.dram_tensor("proj_w", [D, D], F32, kind="ExternalInput").ap()
    projb_d = nc.dram_tensor("proj_b", [D], F32, kind="ExternalInput").ap()
    gamma_d = nc.dram_tensor("qn_gamma", [HD], F32, kind="ExternalInput").ap()
    beta_d = nc.dram_tensor("qn_beta", [HD], F32, kind="ExternalInput").ap()
    out_d = nc.dram_tensor("out", [N, D], F32, kind="ExternalOutput").ap()

    with tile.TileContext(nc) as tc:
        _emit(tc, out_d, x_d, qkvw_d, qkvb_d, projw_d, projb_d, gamma_d, beta_d,
              apply_gn)

    nc.compile()
    return nc


def _emit(tc, out_d, x_d, qkvw_d, qkvb_d, projw_d, projb_d, gamma_d, beta_d,
          apply_gn):
    nc = tc.nc
    ctx = ExitStack()
    with ctx:
        const = ctx.enter_context(tc.tile_pool(name="const", bufs=1))
        wpool = ctx.enter_context(tc.tile_pool(name="wts", bufs=1))
        data = ctx.enter_context(tc.tile_pool(name="data", bufs=1))
        epool = ctx.enter_context(tc.tile_pool(name="escore", bufs=2))
        qkpool = ctx.enter_context(tc.tile_pool(name="qk", bufs=2))
        tmpp = ctx.enter_context(tc.tile_pool(name="tmp", bufs=3))
        stat = ctx.enter_context(tc.tile_pool(name="stat", bufs=4))
        outp = ctx.enter_context(tc.tile_pool(name="outp", bufs=3))
        nrm = ctx.enter_context(tc.tile_pool(name="nrm", bufs=2))

        # ---- constants ----
        qkvb_bc = const.tile([P, O3], F32)
        nc.sync.dma_start(qkvb_bc[:], _bcast_ap(qkvb_d, P))
        projb_bc = const.tile([P, D], F32)
        nc.sync.dma_start(projb_bc[:], _bcast_ap(projb_d, P))
        eps_t = const.tile([P, 1], F32)
        nc.vector.memset(eps_t[:], EPS)
        ident = const.tile([P, P], BF16)
        make_identity(nc, ident[:])
        if apply_gn:
            gamma_bc = const.tile([P, HD], F32)
            nc.sync.dma_start(gamma_bc[:], _bcast_ap(gamma_d, P))
            beta_bc = const.tile([P, HD], F32)
            nc.sync.dma_start(beta_bc[:], _bcast_ap(beta_d, P))

        def pe_transpose_batch(pool, src_tiles, dst, dst_col0, evac_engine):
            """PE-transpose up to 4 [128,128] bf16 tiles through one PSUM bank;
            dst gets columns [dst_col0, dst_col0 + 128*len)."""
            ng = len(src_tiles)
            ps_full = pool.tile([P, 512], BF16, tag="tr", name="ps_tr_t")
            ps = ps_full[:, :ng * P]
            for i, src in enumerate(src_tiles):
                nc.tensor.transpose(ps_full[:, i * P:(i + 1) * P], src, ident[:])
            if evac_engine is nc.scalar:
                evac_engine.copy(dst[:, dst_col0:dst_col0 + ng * P], ps)
            else:
                evac_engine.tensor_copy(dst[:, dst_col0:dst_col0 + ng * P], ps)

        # ---- load + cast + PE-transpose x and weights into [k, ., m] layouts ----
        xT = wpool.tile([P, DC, N], BF16)      # [d_in, d_out, t]
        qkvwT = wpool.tile([P, DC, O3], BF16)  # [d_in, d_out, o]
        projwT = wpool.tile([P, DC, D], BF16)  # [o_in, o_out, e]

        # ---- phase 0+1 PSUM scope: transposes + qkv matmuls ----
        qn = data.tile([P, NT, D], BF16)            # [t_in, t_out, o]  (q heads)
        kn = data.tile([P, NT, D], BF16)
        # v with 64 ones-columns: attn@v psum rows 64:128 become the softmax
        # denominator s[i], broadcast across 64 partitions by the PE for free
        vext = data.tile([P, NT, H, 2 * HD], BF16)
        nc.vector.memset(vext[:, :, :, HD:2 * HD], 1.0)

        with tc.tile_pool(name="ps_tr", bufs=2, space="PSUM") as ps_tr, \
             tc.tile_pool(name="ps_mm", bufs=4, space="PSUM") as ps_mm, \
             tc.tile_pool(name="prep", bufs=2) as prep:

            def load_cast_transpose(src_d, n_rt, dstT):
                for gi, (g0, gn) in enumerate(_groups_of(n_rt, 4)):
                    stage = prep.tile([P, 4, D], BF16, tag="stage", name="stage")
                    for i in range(gn):
                        t_f = prep.tile([P, D], F32, tag="ld_f32", name="t_f")
                        nc.sync.dma_start(
                            t_f[:], src_d[(g0 + i) * P:(g0 + i + 1) * P, :]
                        )
                        nc.vector.tensor_copy(stage[:, i, :], t_f[:])
                    for dc in range(DC):
                        srcs = [stage[:, i, dc * P:(dc + 1) * P]
                                for i in range(gn)]
                        evac = nc.scalar if (gi + dc) % 2 == 0 else nc.vector
                        pe_transpose_batch(ps_tr, srcs, dstT[:, dc, :], g0 * P,
                                           evac)

            load_cast_transpose(x_d, NT, xT)
            load_cast_transpose(qkvw_d, O3 // P, qkvwT)

            # ---- QKV projection + bias + head-dim layernorm on q,k ----
            for tt in range(NT):
                for (c0, cs) in QKV_CHUNKS:
                    psum_full = ps_mm.tile([P, 512], F32, tag="mm", name="psum_mm")
                    psum = psum_full[:, :cs]
                    for dc in range(DC):
                        nc.tensor.matmul(
                            psum,
                            lhsT=xT[:, dc, tt * P:(tt + 1) * P],
                            rhs=qkvwT[:, dc, c0:c0 + cs],
                            start=(dc == 0),
                            stop=(dc == DC - 1),
                        )
                    if c0 < 2 * D:
                        # q/k chunk: bias add then LN over 64-wide segments
                        nsg = cs // HD
                        tmp_c_full = tmpp.tile([P, 512], F32, tag="tmpc", name="tmp_c")
                        tmp_c = tmp_c_full[:, :cs]
                        nc.vector.tensor_add(tmp_c, psum, qkvb_bc[:, c0:c0 + cs])
                        t3 = tmp_c.rearrange("p (s h) -> p s h", h=HD)
                        sums_full = stat.tile([P, 8], F32, tag="sums", name="sums")
                        sums = sums_full[:, :nsg]
                        nc.vector.tensor_reduce(
                            sums, t3, axis=mybir.AxisListType.X,
                            op=mybir.AluOpType.add
                        )
                        sq_full = tmpp.tile([P, 512], F32, tag="sq", name="sq")
                        sq = sq_full[:, :cs]
                        nc.scalar.square(sq, tmp_c)
                        sqs_full = stat.tile([P, 8], F32, tag="sqs", name="sqs")
                        sqs = sqs_full[:, :nsg]
                        nc.vector.tensor_reduce(
                            sqs,
                            sq.rearrange("p (s h) -> p s h", h=HD),
                            axis=mybir.AxisListType.X,
                            op=mybir.AluOpType.add,
                        )
                        mean_full = stat.tile([P, 8], F32, tag="mean", name="mean")
                        mean = mean_full[:, :nsg]
                        nc.vector.tensor_scalar_mul(mean, sums, 1.0 / HD)
                        msq_full = stat.tile([P, 8], F32, tag="msq", name="msq")
                        msq = msq_full[:, :nsg]
                        nc.vector.tensor_mul(msq, mean, mean)
                        var_full = stat.tile([P, 8], F32, tag="var", name="var")
                        var = var_full[:, :nsg]
                        nc.vector.tensor_scalar_mul(var, sqs, 1.0 / HD)
                        nc.vector.tensor_sub(var, var, msq)
                        std_full = stat.tile([P, 8], F32, tag="std", name="std")
                        std = std_full[:, :nsg]
                        nc.scalar.activation(
                            std, var, mybir.ActivationFunctionType.Sqrt,
                            bias=eps_t[:]
                        )
                        rstd_full = stat.tile([P, 8], F32, tag="rstd", name="rstd")
                        rstd = rstd_full[:, :nsg]
                        nc.vector.reciprocal(rstd, std)
                        # normalize: (tmp - mean) * rstd  (broadcast stats over HD)
                        mean_b = mean[:, :, None].to_broadcast((P, nsg, HD))
                        rstd_b = rstd[:, :, None].to_broadcast((P, nsg, HD))
                        nc.gpsimd.tensor_tensor(t3, t3, mean_b,
                                                op=mybir.AluOpType.subtract)
                        if apply_gn:
                            nc.gpsimd.tensor_tensor(t3, t3, rstd_b,
                                                    op=mybir.AluOpType.mult)
                            gamma_b = gamma_bc[:, None, :].to_broadcast((P, nsg, HD))
                            nc.gpsimd.tensor_tensor(t3, t3, gamma_b,
                                                    op=mybir.AluOpType.mult)
                        # write bf16 into qn/kn, splitting at q/k boundary (o=768)
                        spans = []
                        if c0 < D:
                            q_hi = min(c0 + cs, D)
                            spans.append((qn, c0, q_hi - c0, 0))
                        if c0 + cs > D:
                            k_lo = max(c0, D)
                            spans.append((kn, k_lo - D, c0 + cs - k_lo, k_lo - c0))
                        for (dst, d0, dlen, src_off) in spans:
                            nsg_s = dlen // HD
                            src = t3[:, src_off // HD:(src_off + dlen) // HD, :]
                            dgt = dst[:, tt, d0:d0 + dlen].rearrange(
                                "p (s h) -> p s h", h=HD
                            )
                            if apply_gn:
                                beta_b = beta_bc[:, None, :].to_broadcast(
                                    (P, nsg_s, HD))
                                nc.gpsimd.tensor_tensor(
                                    dgt, src, beta_b, op=mybir.AluOpType.add
                                )
                            else:
                                rstd_s = rstd_b[:, src_off // HD:
                                                (src_off + dlen) // HD, :]
                                nc.gpsimd.tensor_tensor(
                                    dgt, src, rstd_s, op=mybir.AluOpType.mult
                                )
                    else:
                        # v chunk: bias add, cast bf16, scatter into vext
                        hs = (c0 - 2 * D) // HD
                        nh = cs // HD
                        nc.vector.tensor_tensor(
                            vext[:, tt, hs:hs + nh, 0:HD],
                            psum.rearrange("p (s h) -> p s h", h=HD),
                            qkvb_bc[:, c0:c0 + cs].rearrange(
                                "p (s h) -> p s h", h=HD),
                            op=mybir.AluOpType.add,
                        )

            # proj_w prep fills the qkv phase's trailing gaps
            load_cast_transpose(projw_d, D // P, projwT)

        # ---- per-head attention ----
        # attnoutT [o_in, o_out, t] is written directly by the normalize step
        attnoutT = data.tile([P, DC, N], BF16)
        with tc.tile_pool(name="ps_tr2", bufs=1, space="PSUM") as ps_tr2, \
             tc.tile_pool(name="ps_st", bufs=3, space="PSUM") as ps_st, \
             tc.tile_pool(name="ps_av", bufs=4, space="PSUM") as ps_av:

            def emit_pair_transposes(hp):
                # kkT: [hd, t] packed pair -- head 2hp in partitions 0:64,
                # 2hp+1 in 64:128.  qp0/qp1: per-head q, zero-padded in the
                # other head's partitions, so scores run at K=128 (full PE
                # array activity keeps the HAM clock-gate warm) with the
                # cross-head products nulled by the zeros.
                kkT = qkpool.tile([P, N], BF16, tag="kkT", name="kkT")
                qp0 = qkpool.tile([P, N], BF16, tag="qp0", name="qp0")
                qp1 = qkpool.tile([P, N], BF16, tag="qp1", name="qp1")
                nc.vector.memset(qp0[HD:2 * HD, :], 0.0)
                nc.vector.memset(qp1[0:HD, :], 0.0)
                for (g0, gn) in _groups_of(NT, 4):
                    srcs_k = [kn[:, g0 + i, hp * P:(hp + 1) * P] for i in range(gn)]
                    pe_transpose_batch(ps_tr2, srcs_k, kkT, g0 * P, nc.vector)
                    ng = gn
                    ps_full = ps_tr2.tile([P, 512], BF16, tag="tr", name="ps_tr_q")
                    for i in range(gn):
                        nc.tensor.transpose(
                            ps_full[:, i * P:(i + 1) * P],
                            qn[:, g0 + i, hp * P:(hp + 1) * P], ident[:])
                    nc.vector.tensor_copy(
                        qp0[0:HD, g0 * P:(g0 + ng) * P], ps_full[0:HD, :ng * P])
                    nc.vector.tensor_copy(
                        qp1[HD:2 * HD, g0 * P:(g0 + ng) * P],
                        ps_full[HD:2 * HD, :ng * P])
                return kkT, qp0, qp1

            def emit_head(h, kkT, qp0, qp1, prev):
                """Emit scores+exp for head h, 1:1 interleaved with the
                attn@v accumulation of head h-1 (prev) so the static TensorE
                stream has ready work during every exp drain."""
                qT = qp0 if h % 2 == 0 else qp1
                E = epool.tile([P, NT, N], BF16, tag="E", name="E")
                if prev is not None:
                    hprev, Eprev = prev
                    pa0 = ps_av.tile([P, 512], F32, tag="av", name="pa0")
                    pa1 = ps_av.tile([P, 512], F32, tag="av", name="pa1")
                for jt in range(NT):
                    for ic in range(2):
                        ps = ps_st.tile([P, 512], F32, tag="st", name="ps_st_t")
                        nc.tensor.matmul(
                            ps,
                            lhsT=kkT[:, jt * P:(jt + 1) * P],
                            rhs=qT[:, ic * 512:(ic + 1) * 512],
                            start=True,
                            stop=True,
                        )
                        nc.scalar.activation(
                            E[:, jt, ic * 512:(ic + 1) * 512],
                            ps,
                            mybir.ActivationFunctionType.Exp,
                            scale=SCALE,
                        )
                    if prev is not None:
                        nc.tensor.matmul(
                            pa0, lhsT=vext[:, jt, hprev, :],
                            rhs=Eprev[:, jt, 0:512],
                            start=(jt == 0), stop=(jt == NT - 1),
                        )
                        nc.tensor.matmul(
                            pa1, lhsT=vext[:, jt, hprev, :],
                            rhs=Eprev[:, jt, 512:1024],
                            start=(jt == 0), stop=(jt == NT - 1),
                        )
                if prev is not None:
                    emit_normalize(hprev, pa0, pa1)
                return E

            def emit_av_tail(h, E):
                pa0 = ps_av.tile([P, 512], F32, tag="av", name="pa0")
                pa1 = ps_av.tile([P, 512], F32, tag="av", name="pa1")
                for jt in range(NT):
                    nc.tensor.matmul(
                        pa0, lhsT=vext[:, jt, h, :], rhs=E[:, jt, 0:512],
                        start=(jt == 0), stop=(jt == NT - 1),
                    )
                    nc.tensor.matmul(
                        pa1, lhsT=vext[:, jt, h, :], rhs=E[:, jt, 512:1024],
                        start=(jt == 0), stop=(jt == NT - 1),
                    )
                emit_normalize(h, pa0, pa1)

            def emit_normalize(h, pa0, pa1):
                for ic, pa in ((0, pa0), (1, pa1)):
                    s_sb = nrm.tile([HD, 512], F32, tag="s_sb", name="s_sb")
                    nc.vector.tensor_copy(s_sb[:], pa[HD:2 * HD, :])
                    rcp_t = nrm.tile([HD, 512], F32, tag="rcp_t", name="rcp_t")
                    nc.vector.reciprocal_approx_fast(rcp_t[:], s_sb[:])
                    nc.vector.tensor_tensor(
                        attnoutT[(h % 2) * HD:(h % 2 + 1) * HD, h // 2,
                                 ic * 512:(ic + 1) * 512],
                        pa[0:HD, :],
                        rcp_t[:],
                        op=mybir.AluOpType.mult,
                    )

            cur = emit_pair_transposes(0)
            prev = None  # (h, E)
            for h in range(H):
                hp, hh = divmod(h, 2)
                if hh == 0 and hp > 0:
                    cur = nxt
                E = emit_head(h, *cur, prev)
                if hh == 1 and hp + 1 < H // 2:
                    nxt = emit_pair_transposes(hp + 1)
                prev = (h, E)
            emit_av_tail(*prev)

        # ---- output projection ----
        EC = 384
        with tc.tile_pool(name="ps_pj", bufs=2, space="PSUM") as ps_pj:
            for tt in range(NT):
                for ec in range(D // EC):
                    ps_full = ps_pj.tile([P, 512], F32, tag="mm", name="ps_proj")
                    ps = ps_full[:, :EC]
                    for oc in range(DC):
                        nc.tensor.matmul(
                            ps,
                            lhsT=attnoutT[:, oc, tt * P:(tt + 1) * P],
                            rhs=projwT[:, oc, ec * EC:(ec + 1) * EC],
                            start=(oc == 0),
                            stop=(oc == DC - 1),
                        )
                    ot = outp.tile([P, EC], F32, tag="outt")
                    nc.vector.tensor_add(ot[:], ps,
                                         projb_bc[:, ec * EC:(ec + 1) * EC])
                    nc.sync.dma_start(
                        out_d[tt * P:(tt + 1) * P, ec * EC:(ec + 1) * EC], ot[:]
                    )


_NC_CACHE = {}


def _get_nc(apply_gn=True):
    if apply_gn not in _NC_CACHE:
        _NC_CACHE[apply_gn] = _build_graph(apply_gn)
    return _NC_CACHE[apply_gn]


def kernel(x, qkv_w, qkv_b, proj_w, proj_b, qn_gamma, qn_beta):
    qn_gamma = np.ascontiguousarray(qn_gamma, np.float32)
    qn_beta = np.ascontiguousarray(qn_beta, np.float32)
    apply_gn = not (np.all(qn_gamma == 1.0) and np.all(qn_beta == 0.0))
    nc = _get_nc(apply_gn)
    shared = {
        "qkv_w": np.ascontiguousarray(qkv_w, np.float32),
        "qkv_b": np.ascontiguousarray(qkv_b, np.float32),
        "proj_w": np.ascontiguousarray(proj_w, np.float32),
        "proj_b": np.ascontiguousarray(proj_b, np.float32),
        "qn_gamma": qn_gamma,
        "qn_beta": qn_beta,
    }
    in_maps = [
        {**shared, "x": np.ascontiguousarray(x[i], np.float32)} for i in range(B)
    ]
    res = run_bass_kernel_spmd(nc, in_maps, core_ids=list(range(B)))
    return np.stack([res.results[i]["out"] for i in range(B)], axis=0)


# revision 19
# speedup vs baseline: 1.6460x; 1.1768x over previous
"""Multi-head attention forward on 8 TRN2 NeuronCores, data-parallel over batch.

Reference computation (per batch element b):
    qkv  = x @ qkv_w.T + qkv_b                     # [N, 3D]
    q, k = LN_headdim(q), LN_headdim(k)            # layernorm over head_dim=64
    S    = q @ k.T * hd^-0.5 ; A = softmax_j(S)    # per head
    out  = (A @ v) @ proj_w.T + proj_b             # [N, D]

Kernel strategy (one batch element per core, no collectives):
  - bf16 matmuls on TensorE; f32 statistics/softmax denominators.
  - Scores computed TRANSPOSED: ST[j,i] = k_j . q_i so that E = exp(ST*scale)
    lands in SBUF with the contraction axis j on partitions -- E is directly
    the lhsT of the attn@v matmul (no attention-matrix transpose needed).
  - Softmax denominators come free: V gets a ones-column appended, so
    psum[i, 64] = sum_j E[j,i]; normalize with a per-partition scalar.
  - No max-subtraction in softmax: q,k are layernormed so |q.k|*scale <= 8,
    exp() is safely bounded (<= e^8) in f32/bf16.
  - All transposes on TensorE (identity matmul), batched 4 tiles into one
    [128,512] PSUM bank with a single evacuation copy.  DMA transposes are
    avoided entirely: they shatter into 256B packets (measured 780us of DMA
    engine time for this problem).
  - Engine balance: exp on ScalarE; reductions/psum-reads on VectorE;
    SBUF-only elementwise (casts, squares, LN scale) on GpSimd.
"""

import sys

import numpy as np

sys.path.insert(0, "/opt/trn_rl_repo")

from contextlib import ExitStack

import concourse.bass as bass
import concourse.tile as tile
from concourse import bacc, mybir
from concourse.bass_utils import run_bass_kernel_spmd
from concourse.masks import make_identity

B, N, D = 8, 1024, 768
H, HD = 12, 64
O3 = 3 * D  # 2304
P = 128
NT = N // P  # 8 token tiles
DC = D // P  # 6 contraction subtiles
EPS = 1e-5
SCALE = HD ** -0.5  # 0.125
F32 = mybir.dt.float32
BF16 = mybir.dt.bfloat16

# qkv output chunks: [start, size]; q = o[0:768), k = [768:1536), v = [1536:2304)
QKV_CHUNKS = [(0, 512), (512, 512), (1024, 512), (1536, 512), (2048, 256)]


def _bcast_ap(ap_1d, parts):
    """View a 1-D DRAM AP as [parts, n] with partition stride 0 (broadcast)."""
    return bass.AP(
        tensor=ap_1d.tensor,
        offset=ap_1d.offset,
        ap=[[0, parts]] + list(ap_1d.ap),
    )


def _groups_of(n, g):
    """Split range(n) into [(start, len)] groups of at most g."""
    return [(s, min(g, n - s)) for s in range(0, n, g)]


def _build_graph(apply_gn):
    nc = bacc.Bacc("TRN2", target_bir_lowering=False, debug=False, num_devices=B)

    x_d = nc.dram_tensor("x", [N, D], F32, kind="ExternalInput").ap()
    qkvw_d = nc.dram_tensor("qkv_w", [O3, D], F32, kind="ExternalInput").ap()
    qkvb_d = nc.dram_tensor("qkv_b", [O3], F32, kind="ExternalInput").ap()
    projw_d = nc.dram_tensor("proj_w", [D, D], F32, kind="ExternalInput").ap()
    projb_d = nc.dram_tensor("proj_b", [D], F32, kind="ExternalInput").ap()
    gamma_d = nc.dram_tensor("qn_gamma", [HD], F32, kind="ExternalInput").ap()
    beta_d = nc.dram_tensor("qn_beta", [HD], F32, kind="ExternalInput").ap()
    out_d = nc.dram_tensor("out", [N, D], F32, kind="ExternalOutput").ap()

    with tile.TileContext(nc) as tc:
        _emit(tc, out_d, x_d, qkvw_d, qkvb_d, projw_d, projb_d, gamma_d, beta_d,
              apply_gn)

    nc.compile()
    return nc


def _emit(tc, out_d, x_d, qkvw_d, qkvb_d, projw_d, projb_d, gamma_d, beta_d,
          apply_gn):
    nc = tc.nc
    ctx = ExitStack()
    with ctx:
        const = ctx.enter_context(tc.tile_pool(name="const", bufs=1))
        wpool = ctx.enter_context(tc.tile_pool(name="wts", bufs=1))
        data = ctx.enter_context(tc.tile_pool(name="data", bufs=1))
        epool = ctx.enter_context(tc.tile_pool(name="escore", bufs=2))
        qkpool = ctx.enter_context(tc.tile_pool(name="qk", bufs=2))
        tmpp = ctx.enter_context(tc.tile_pool(name="tmp", bufs=3))
        stat = ctx.enter_context(tc.tile_pool(name="stat", bufs=4))
        outp = ctx.enter_context(tc.tile_pool(name="outp", bufs=3))
        nrm = ctx.enter_context(tc.tile_pool(name="nrm", bufs=2))

        # ---- constants ----
        qkvb_bc = const.tile([P, O3], F32)
        nc.sync.dma_start(qkvb_bc[:], _bcast_ap(qkvb_d, P))
        projb_bc = const.tile([P, D], F32)
        nc.sync.dma_start(projb_bc[:], _bcast_ap(projb_d, P))
        eps_t = const.tile([P, 1], F32)
        nc.vector.memset(eps_t[:], EPS)
        ident = const.tile([P, P], BF16)
        make_identity(nc, ident[:])
        ident32 = const.tile([P, P], F32)
        make_identity(nc, ident32[:])
        if apply_gn:
            gamma_bc = const.tile([P, HD], F32)
            nc.sync.dma_start(gamma_bc[:], _bcast_ap(gamma_d, P))
            beta_bc = const.tile([P, HD], F32)
            nc.sync.dma_start(beta_bc[:], _bcast_ap(beta_d, P))

        def pe_transpose_batch(pool, src_tiles, dst, dst_col0, evac_engine):
            """PE-transpose up to 4 [128,128] tiles through one PSUM bank;
            dst gets columns [dst_col0, dst_col0 + 128*len).  f32 sources cast
            to the (bf16) dst during the evacuation copy."""
            ng = len(src_tiles)
            dt = src_tiles[0].dtype
            idn = ident32 if dt == F32 else ident
            ps_full = pool.tile([P, 512], dt, tag="tr", name="ps_tr_t")
            ps = ps_full[:, :ng * P]
            for i, src in enumerate(src_tiles):
                nc.tensor.transpose(ps_full[:, i * P:(i + 1) * P], src, idn[:])
            if evac_engine is nc.scalar:
                evac_engine.copy(dst[:, dst_col0:dst_col0 + ng * P], ps)
            else:
                evac_engine.tensor_copy(dst[:, dst_col0:dst_col0 + ng * P], ps)

        # ---- load + cast + PE-transpose x and weights into [k, ., m] layouts ----
        xT = wpool.tile([P, DC, N], BF16)      # [d_in, d_out, t]
        qkvwT = wpool.tile([P, DC, O3], BF16)  # [d_in, d_out, o]
        projwT = wpool.tile([P, DC, D], BF16)  # [o_in, o_out, e]

        # ---- phase 0+1 PSUM scope: transposes + qkv matmuls ----
        qn = data.tile([P, NT, D], BF16)            # [t_in, t_out, o]  (q heads)
        kn = data.tile([P, NT, D], BF16)
        # v with 64 ones-columns: attn@v psum rows 64:128 become the softmax
        # denominator s[i], broadcast across 64 partitions by the PE for free
        vext = data.tile([P, NT, H, 2 * HD], BF16)
        nc.vector.memset(vext[:, :, :, HD:2 * HD], 1.0)

        with tc.tile_pool(name="ps_tr", bufs=2, space="PSUM") as ps_tr, \
             tc.tile_pool(name="ps_mm", bufs=4, space="PSUM") as ps_mm, \
             tc.tile_pool(name="prep", bufs=5) as prep:

            def load_cast_transpose(src_d, n_rt, dstT):
                for gi, (g0, gn) in enumerate(_groups_of(n_rt, 4)):
                    tfs = []
                    for i in range(gn):
                        t_f = prep.tile([P, D], F32, tag="ld_f32", name="t_f")
                        nc.sync.dma_start(
                            t_f[:], src_d[(g0 + i) * P:(g0 + i + 1) * P, :]
                        )
                        tfs.append(t_f)
                    for dc in range(DC):
                        srcs = [tfs[i][:, dc * P:(dc + 1) * P]
                                for i in range(gn)]
                        evac = nc.scalar if (gi + dc) % 2 == 0 else nc.vector
                        pe_transpose_batch(ps_tr, srcs, dstT[:, dc, :], g0 * P,
                                           evac)

            load_cast_transpose(x_d, NT, xT)
            load_cast_transpose(qkvw_d, O3 // P, qkvwT)

            # ---- QKV projection + bias + head-dim layernorm on q,k ----
            for (c0, cs) in QKV_CHUNKS:
                for tt in range(NT):
                    psum_full = ps_mm.tile([P, 512], F32, tag="mm", name="psum_mm")
                    psum = psum_full[:, :cs]
                    for dc in range(DC):
                        nc.tensor.matmul(
                            psum,
                            lhsT=xT[:, dc, tt * P:(tt + 1) * P],
                            rhs=qkvwT[:, dc, c0:c0 + cs],
                            start=(dc == 0),
                            stop=(dc == DC - 1),
                        )
                    if c0 < 2 * D:
                        # q/k chunk: bias add then LN over 64-wide segments
                        nsg = cs // HD
                        tmp_c_full = tmpp.tile([P, 512], F32, tag="tmpc", name="tmp_c")
                        tmp_c = tmp_c_full[:, :cs]
                        nc.vector.tensor_add(tmp_c, psum, qkvb_bc[:, c0:c0 + cs])
                        t3 = tmp_c.rearrange("p (s h) -> p s h", h=HD)
                        sums_full = stat.tile([P, 8], F32, tag="sums", name="sums")
                        sums = sums_full[:, :nsg]
                        nc.vector.tensor_reduce(
                            sums, t3, axis=mybir.AxisListType.X,
                            op=mybir.AluOpType.add
                        )
                        sq_full = tmpp.tile([P, 512], F32, tag="sq", name="sq")
                        sq = sq_full[:, :cs]
                        nc.scalar.square(sq, tmp_c)
                        sqs_full = stat.tile([P, 8], F32, tag="sqs", name="sqs")
                        sqs = sqs_full[:, :nsg]
                        nc.vector.tensor_reduce(
                            sqs,
                            sq.rearrange("p (s h) -> p s h", h=HD),
                            axis=mybir.AxisListType.X,
                            op=mybir.AluOpType.add,
                        )
                        mean_full = stat.tile([P, 8], F32, tag="mean", name="mean")
                        mean = mean_full[:, :nsg]
                        nc.vector.tensor_scalar_mul(mean, sums, 1.0 / HD)
                        msq_full = stat.tile([P, 8], F32, tag="msq", name="msq")
                        msq = msq_full[:, :nsg]
                        nc.vector.tensor_mul(msq, mean, mean)
                        var_full = stat.tile([P, 8], F32, tag="var", name="var")
                        var = var_full[:, :nsg]
                        nc.vector.tensor_scalar_mul(var, sqs, 1.0 / HD)
                        nc.vector.tensor_sub(var, var, msq)
                        std_full = stat.tile([P, 8], F32, tag="std", name="std")
                        std = std_full[:, :nsg]
                        nc.scalar.activation(
                            std, var, mybir.ActivationFunctionType.Sqrt,
                            bias=eps_t[:]
                        )
                        rstd_full = stat.tile([P, 8], F32, tag="rstd", name="rstd")
                        rstd = rstd_full[:, :nsg]
                        nc.vector.reciprocal(rstd, std)
                        # normalize: (tmp - mean) * rstd  (broadcast stats over HD)
                        mean_b = mean[:, :, None].to_broadcast((P, nsg, HD))
                        rstd_b = rstd[:, :, None].to_broadcast((P, nsg, HD))
                        nc.gpsimd.tensor_tensor(t3, t3, mean_b,
                                                op=mybir.AluOpType.subtract)
                        if apply_gn:
                            nc.gpsimd.tensor_tensor(t3, t3, rstd_b,
                                                    op=mybir.AluOpType.mult)
                            gamma_b = gamma_bc[:, None, :].to_broadcast((P, nsg, HD))
                            nc.gpsimd.tensor_tensor(t3, t3, gamma_b,
                                                    op=mybir.AluOpType.mult)
                        # write bf16 into qn/kn, splitting at q/k boundary (o=768)
                        spans = []
                        if c0 < D:
                            q_hi = min(c0 + cs, D)
                            spans.append((qn, c0, q_hi - c0, 0))
                        if c0 + cs > D:
                            k_lo = max(c0, D)
                            spans.append((kn, k_lo - D, c0 + cs - k_lo, k_lo - c0))
                        for (dst, d0, dlen, src_off) in spans:
                            nsg_s = dlen // HD
                            src = t3[:, src_off // HD:(src_off + dlen) // HD, :]
                            dgt = dst[:, tt, d0:d0 + dlen].rearrange(
                                "p (s h) -> p s h", h=HD
                            )
                            if apply_gn:
                                beta_b = beta_bc[:, None, :].to_broadcast(
                                    (P, nsg_s, HD))
                                nc.gpsimd.tensor_tensor(
                                    dgt, src, beta_b, op=mybir.AluOpType.add
                                )
                            else:
                                rstd_s = rstd_b[:, src_off // HD:
                                                (src_off + dlen) // HD, :]
                                nc.gpsimd.tensor_tensor(
                                    dgt, src, rstd_s, op=mybir.AluOpType.mult
                                )
                    else:
                        # v chunk: bias add, cast bf16, scatter into vext
                        hs = (c0 - 2 * D) // HD
                        nh = cs // HD
                        nc.vector.tensor_tensor(
                            vext[:, tt, hs:hs + nh, 0:HD],
                            psum.rearrange("p (s h) -> p s h", h=HD),
                            qkvb_bc[:, c0:c0 + cs].rearrange(
                                "p (s h) -> p s h", h=HD),
                            op=mybir.AluOpType.add,
                        )

            # proj_w prep fills the qkv phase's trailing gaps
            load_cast_transpose(projw_d, D // P, projwT)

        # ---- per-head attention ----
        # attnoutT [o_in, o_out, t] is written directly by the normalize step
        attnoutT = data.tile([P, DC, N], BF16)
        qp_sets = []
        for si in range(2):
            q0 = data.tile([P, N], BF16, tag=f"qp0_{si}", name="qp0p")
            q1 = data.tile([P, N], BF16, tag=f"qp1_{si}", name="qp1p")
            nc.vector.memset(q0[HD:2 * HD, :], 0.0)
            nc.vector.memset(q1[0:HD, :], 0.0)
            qp_sets.append((q0, q1))
        with tc.tile_pool(name="ps_tr2", bufs=1, space="PSUM") as ps_tr2, \
             tc.tile_pool(name="ps_st", bufs=3, space="PSUM") as ps_st, \
             tc.tile_pool(name="ps_av", bufs=4, space="PSUM") as ps_av:

            def emit_pair_transposes(hp):
                # kkT: [hd, t] packed pair -- head 2hp in partitions 0:64,
                # 2hp+1 in 64:128.  qp0/qp1: per-head q, zero-padded in the
                # other head's partitions, so scores run at K=128 (full PE
                # array activity keeps the HAM clock-gate warm) with the
                # cross-head products nulled by the zeros.
                kkT = qkpool.tile([P, N], BF16, tag="kkT", name="kkT")
                qp0, qp1 = qp_sets[hp % 2]
                for (g0, gn) in _groups_of(NT, 4):
                    srcs_k = [kn[:, g0 + i, hp * P:(hp + 1) * P] for i in range(gn)]
                    pe_transpose_batch(ps_tr2, srcs_k, kkT, g0 * P, nc.vector)
                    ng = gn
                    ps_full = ps_tr2.tile([P, 512], BF16, tag="tr", name="ps_tr_q")
                    for i in range(gn):
                        nc.tensor.transpose(
                            ps_full[:, i * P:(i + 1) * P],
                            qn[:, g0 + i, hp * P:(hp + 1) * P], ident[:])
                    nc.vector.tensor_copy(
                        qp0[0:HD, g0 * P:(g0 + ng) * P], ps_full[0:HD, :ng * P])
                    nc.vector.tensor_copy(
                        qp1[HD:2 * HD, g0 * P:(g0 + ng) * P],
                        ps_full[HD:2 * HD, :ng * P])
                return kkT, qp0, qp1

            def emit_head(h, kkT, qp0, qp1, prev):
                """Emit scores+exp for head h, 1:1 interleaved with the
                attn@v accumulation of head h-1 (prev) so the static TensorE
                stream has ready work during every exp drain."""
                qT = qp0 if h % 2 == 0 else qp1
                E = epool.tile([P, NT, N], BF16, tag="E", name="E")
                if prev is not None:
                    hprev, Eprev = prev
                    pa0 = ps_av.tile([P, 512], F32, tag="av", name="pa0")
                    pa1 = ps_av.tile([P, 512], F32, tag="av", name="pa1")
                for jt in range(NT):
                    for ic in range(2):
                        ps = ps_st.tile([P, 512], F32, tag="st", name="ps_st_t")
                        nc.tensor.matmul(
                            ps,
                            lhsT=kkT[:, jt * P:(jt + 1) * P],
                            rhs=qT[:, ic * 512:(ic + 1) * 512],
                            start=True,
                            stop=True,
                        )
                        nc.scalar.activation(
                            E[:, jt, ic * 512:(ic + 1) * 512],
                            ps,
                            mybir.ActivationFunctionType.Exp,
                            scale=SCALE,
                        )
                    if prev is not None:
                        nc.tensor.matmul(
                            pa0, lhsT=vext[:, jt, hprev, :],
                            rhs=Eprev[:, jt, 0:512],
                            start=(jt == 0), stop=(jt == NT - 1),
                        )
                        nc.tensor.matmul(
                            pa1, lhsT=vext[:, jt, hprev, :],
                            rhs=Eprev[:, jt, 512:1024],
                            start=(jt == 0), stop=(jt == NT - 1),
                        )
                if prev is not None:
                    emit_normalize(hprev, pa0, pa1)
                return E

            def emit_av_tail(h, E):
                pa0 = ps_av.tile([P, 512], F32, tag="av", name="pa0")
                pa1 = ps_av.tile([P, 512], F32, tag="av", name="pa1")
                for jt in range(NT):
                    nc.tensor.matmul(
                        pa0, lhsT=vext[:, jt, h, :], rhs=E[:, jt, 0:512],
                        start=(jt == 0), stop=(jt == NT - 1),
                    )
                    nc.tensor.matmul(
                        pa1, lhsT=vext[:, jt, h, :], rhs=E[:, jt, 512:1024],
                        start=(jt == 0), stop=(jt == NT - 1),
                    )
                emit_normalize(h, pa0, pa1)

            def emit_normalize(h, pa0, pa1):
                for ic, pa in ((0, pa0), (1, pa1)):
                    s_sb = nrm.tile([HD, 512], F32, tag="s_sb", name="s_sb")
                    nc.vector.tensor_copy(s_sb[:], pa[HD:2 * HD, :])
                    rcp_t = nrm.tile([HD, 512], F32, tag="rcp_t", name="rcp_t")
                    nc.vector.reciprocal_approx_fast(rcp_t[:], s_sb[:])
                    nc.vector.tensor_tensor(
                        attnoutT[(h % 2) * HD:(h % 2 + 1) * HD, h // 2,
                                 ic * 512:(ic + 1) * 512],
                        pa[0:HD, :],
                        rcp_t[:],
                        op=mybir.AluOpType.mult,
                    )

            cur = emit_pair_transposes(0)
            prev = None  # (h, E)
            for h in range(H):
                hp, hh = divmod(h, 2)
                if hh == 0 and hp > 0:
                    cur = nxt
                E = emit_head(h, *cur, prev)
                if hh == 1 and hp + 1 < H // 2:
                    nxt = emit_pair_transposes(hp + 1)
                prev = (h, E)
            emit_av_tail(*prev)

        # ---- output projection ----
        EC = 384
        with tc.tile_pool(name="ps_pj", bufs=2, space="PSUM") as ps_pj:
            for tt in range(NT):
                for ec in range(D // EC):
                    ps_full = ps_pj.tile([P, 512], F32, tag="mm", name="ps_proj")
                    ps = ps_full[:, :EC]
                    for oc in range(DC):
                        nc.tensor.matmul(
                            ps,
                            lhsT=attnoutT[:, oc, tt * P:(tt + 1) * P],
                            rhs=projwT[:, oc, ec * EC:(ec + 1) * EC],
                            start=(oc == 0),
                            stop=(oc == DC - 1),
                        )
                    ot = outp.tile([P, EC], F32, tag="outt")
                    nc.vector.tensor_add(ot[:], ps,
                                         projb_bc[:, ec * EC:(ec + 1) * EC])
                    nc.sync.dma_start(
                        out_d[tt * P:(tt + 1) * P, ec * EC:(ec + 1) * EC], ot[:]
                    )


_NC_CACHE = {}


def _get_nc(apply_gn=True):
    if apply_gn not in _NC_CACHE:
        _NC_CACHE[apply_gn] = _build_graph(apply_gn)
    return _NC_CACHE[apply_gn]


def kernel(x, qkv_w, qkv_b, proj_w, proj_b, qn_gamma, qn_beta):
    qn_gamma = np.ascontiguousarray(qn_gamma, np.float32)
    qn_beta = np.ascontiguousarray(qn_beta, np.float32)
    apply_gn = not (np.all(qn_gamma == 1.0) and np.all(qn_beta == 0.0))
    nc = _get_nc(apply_gn)
    shared = {
        "qkv_w": np.ascontiguousarray(qkv_w, np.float32),
        "qkv_b": np.ascontiguousarray(qkv_b, np.float32),
        "proj_w": np.ascontiguousarray(proj_w, np.float32),
        "proj_b": np.ascontiguousarray(proj_b, np.float32),
        "qn_gamma": qn_gamma,
        "qn_beta": qn_beta,
    }
    in_maps = [
        {**shared, "x": np.ascontiguousarray(x[i], np.float32)} for i in range(B)
    ]
    res = run_bass_kernel_spmd(nc, in_maps, core_ids=list(range(B)))
    return np.stack([res.results[i]["out"] for i in range(B)], axis=0)
